# revision 26
# baseline (speedup 1.0000x reference)
"""AdaptiveAttention kernel for 8 trn2 NeuronCores — Bass/Tile implementation.

Strategy (per sharding hint): data-parallel over batch B=8, one batch element
per NeuronCore; weights replicated. Host-side preprocessing (cheap, per-token
independent): interpolate the pos table for the given resolution, add it to x,
and store x+pos transposed as bf16 so the device loads matmul lhsT tiles
directly. Device per core:

  1. qkv = (x+pos) @ w_qkv + b_qkv    -- PE matmuls (bf16)
  2. per-token 8x8 head attention     -- DVE broadcast-product + packed-2-byte
                                         tree reduction (TensorReduce has no
                                         2x mode, packed adds do), exp on ACT
  3. scrambled flatten ([N,H,D] -> transpose(1,2).reshape(N,C)) -- folded
     into a strided DMA through a DRAM bounce buffer
  4. out = y @ w_proj + b_proj        -- XBAR transpose-load of y, PE matmuls

Hardcoded problem shape: B=8, N=4096, C=512, H=8, D=64.
"""

import hashlib
import os
import sys

import numpy as np

# Persistent XLA/NEFF compilation cache: a fresh process reuses the compiled
# executable instead of re-running the Neuron compiler (~8 s).
os.environ.setdefault("JAX_COMPILATION_CACHE_DIR", "/tmp/jax_comp_cache")
os.environ.setdefault("JAX_PERSISTENT_CACHE_MIN_COMPILE_TIME_SECS", "1")

for _p in ("/opt/trn_rl_repo",):
    if _p not in sys.path:
        sys.path.append(_p)

B, N, C = 8, 4096, 512
H = 8
D = C // H  # 64
NG = 4          # token groups per core
GTOK = N // NG  # 1024 tokens per group
TT = GTOK // 128  # 8 token-tiles per group

_STATE = {}


def _interp_linear_np(pos, out_len):
    # F.interpolate(mode='linear', align_corners=False) along axis 1.
    in_len = pos.shape[1]
    if in_len == out_len:
        return pos
    scale = in_len / out_len
    coords = (np.arange(out_len, dtype=np.float64) + 0.5) * scale - 0.5
    coords = np.clip(coords, 0.0, in_len - 1)
    i0 = np.floor(coords).astype(np.int64)
    i1 = np.minimum(i0 + 1, in_len - 1)
    w = (coords - i0).astype(np.float32)[None, :, None]
    return pos[:, i0, :] * (1.0 - w) + pos[:, i1, :] * w


def _split_excess_waits(nc, mybir):
    """Walrus codegen in this container accepts at most one sync-wait per
    instruction; hoist extra waits into standalone EventSemaphore
    instructions on the same engine queue (executed in order, so semantics
    are preserved)."""
    n_split = 0
    for fn in nc.m.functions:
        for blk in fn.blocks:
            insts = blk.instructions
            out = []
            changed = False
            for inst in insts:
                si = inst.sync_info
                if si is not None and len(si.on_wait) > 1:
                    for i, w in enumerate(si.on_wait[:-1]):
                        ev = mybir.InstEventSemaphore(
                            name=f"{inst.name}-wsplit{i}"
                        )
                        ev.engine = inst.engine
                        ev.sync_info = mybir.SyncInfo(on_wait=[w], on_update=[])
                        out.append(ev)
                    inst.sync_info = mybir.SyncInfo(
                        on_wait=[si.on_wait[-1]], on_update=list(si.on_update)
                    )
                    out.append(inst)
                    changed = True
                    n_split += 1
                else:
                    out.append(inst)
            if changed:
                blk.instructions = out
    return n_split


def _build_nc(split_waits=True, with_bias=False):
    from contextlib import ExitStack

    import concourse.bass as bass
    import concourse.mybir as mybir
    import concourse.tile as tile
    from concourse.tile import add_dep_helper

    dt = mybir.dt
    bf16 = dt.bfloat16
    f16 = dt.float16
    f32 = dt.float32
    AX = mybir.AxisListType
    ALU = mybir.AluOpType
    ACTF = mybir.ActivationFunctionType

    nc = bass.Bass(num_swdge_queues=4)
    xpT = nc.declare_dram_parameter("xpT", [C, N], bf16, isOutput=False)
    wqkv = nc.declare_dram_parameter("wqkv", [C, 3 * C], bf16, isOutput=False)
    bqkv = nc.declare_dram_parameter("bqkv", [1, 3 * C], bf16, isOutput=False)
    wproj = nc.declare_dram_parameter("wproj", [C, C], bf16, isOutput=False)
    bproj = nc.declare_dram_parameter("bproj", [1, C], bf16, isOutput=False)
    out = nc.declare_dram_parameter("out", [N, C], bf16, isOutput=True)

    with tile.TileContext(nc) as tc, ExitStack() as ctx:
        const = ctx.enter_context(tc.tile_pool(name="const", bufs=1))
        xptp = ctx.enter_context(tc.tile_pool(name="xptp", bufs=3))
        qkvp = ctx.enter_context(tc.tile_pool(name="qkvp", bufs=4))
        prodp = ctx.enter_context(tc.tile_pool(name="prodp", bufs=4))
        treep = ctx.enter_context(tc.tile_pool(name="treep", bufs=3))
        smallp = ctx.enter_context(tc.tile_pool(name="smallp", bufs=4))
        outp = ctx.enter_context(tc.tile_pool(name="outp", bufs=3))
        ytp = ctx.enter_context(tc.tile_pool(name="ytp", bufs=8))
        psump = ctx.enter_context(tc.tile_pool(name="psump", bufs=2, space="PSUM"))
        dramp = ctx.enter_context(tc.tile_pool(name="dramp", bufs=2, space="DRAM"))

        # Replicated weights in SBUF. [128, cc, f] with cc = contraction chunk.
        wq = const.tile([128, 4, 3 * C], bf16)
        nc.scalar.dma_start(out=wq, in_=wqkv.rearrange("(cc p) f -> p cc f", p=128))
        wp = const.tile([128, 4, C], bf16)
        nc.scalar.dma_start(out=wp, in_=wproj.rearrange("(cc p) f -> p cc f", p=128))
        if with_bias:
            # Biases broadcast to all partitions (DVE cannot partition-bcast).
            bq = const.tile([128, 3 * C], bf16)
            nc.scalar.dma_start(out=bq, in_=bqkv[:].to_broadcast((128, 3 * C)))
            bp = const.tile([128, C], bf16)
            nc.scalar.dma_start(out=bp, in_=bproj[:].to_broadcast((128, C)))

        def emit_proj(yg_s, g_s, h):
            # One output row-tile: rows h*512+128*g_s .. +128.
            yt = ytp.tile([128, 4, 128], bf16, tag="yt")
            # One blocked XBAR transpose: [128, 512] -> [512, 128] delivered
            # as 4 partition blocks stacked on the free axis.
            nc.sync.dma_start_transpose(yt, yg_s[h])
            pso = psump.tile([128, C], f32, tag="proj_ps")
            for cc in range(4):
                nc.tensor.matmul(
                    pso,
                    lhsT=yt[:, cc, :],
                    rhs=wp[:, cc, :],
                    start=(cc == 0),
                    stop=(cc == 3),
                )
            osb = outp.tile([128, C], bf16, tag="osb")
            if with_bias:
                nc.vector.tensor_add(osb, pso, bp)
            else:
                osb_i = nc.scalar.activation(osb, pso, ACTF.Copy)
                if prev_exp[0] is not None:
                    add_dep_helper(
                        osb_i.ins, prev_exp[0].ins, sync=False,
                        reason="exp before proj copy on ACT",
                    )
            nc.gpsimd.dma_start(
                out=out[h * 512 + 128 * g_s : h * 512 + 128 * (g_s + 1), :],
                in_=osb,
            )

        yg_prev = None
        prev_exp = [None]
        for g in range(NG):
            # x+pos transposed slab for this group: [c-part, cc, token]
            xpt = xptp.tile([128, 4, GTOK], bf16, tag="xpt")
            nc.gpsimd.dma_start(
                out=xpt,
                in_=xpT.rearrange("(cc p) n -> p cc n", p=128)[
                    :, :, g * GTOK : (g + 1) * GTOK
                ],
            )
            # Scrambled attention output bounce buffer (y rows for this group).
            yg = dramp.tile([H, 128, C], bf16, tag="yg")

            for k in range(TT):
                # --- qkv matmul: [128 tok, 1536] into one 3-bank PSUM tile ---
                ps = psump.tile([128, 3 * C], f32, tag="qkv_ps")
                for oc in range(3):
                    for cc in range(4):
                        nc.tensor.matmul(
                            ps[:, oc * 512 : (oc + 1) * 512],
                            lhsT=xpt[:, cc, k * 128 : (k + 1) * 128],
                            rhs=wq[:, cc, oc * 512 : (oc + 1) * 512],
                            start=(cc == 0),
                            stop=(cc == 3),
                        )
                # q,k kept in token-major layout; v transposed to (d, j) so
                # the AV product streams are packed along j.
                qk = qkvp.tile([128, 2 * C], bf16, tag="qk")
                vt = qkvp.tile([128, C], bf16, tag="vt")
                vt_jd = vt.rearrange("p (d j) -> p j d", d=D)
                vsrc = ps[:, 2 * C : 3 * C].rearrange("p (j d) -> p j d", j=H)
                if with_bias:
                    nc.vector.tensor_add(qk, ps[:, 0 : 2 * C], bq[:, 0 : 2 * C])
                    nc.vector.tensor_add(
                        vt_jd,
                        vsrc,
                        bq[:, 2 * C : 3 * C].rearrange("p (j d) -> p j d", j=H),
                    )
                else:
                    # ACT is nearly idle; offload the plain copies there.
                    qk_i = nc.scalar.activation(qk, ps[:, 0 : 2 * C], ACTF.Copy)
                    nc.scalar.activation(vt_jd, vsrc, ACTF.Copy)
                    if prev_exp[0] is not None:
                        # Keep the previous tile's exp ahead of this tile's
                        # PE-gated copies in the in-order ACT queue.
                        add_dep_helper(
                            qk_i.ins, prev_exp[0].ins, sync=False,
                            reason="exp before next-tile ACT copies",
                        )

                # --- per-token 8x8 head attention ---
                q = qk[:, 0:C].rearrange("p (i d) -> p i d", i=H)
                kk = qk[:, C : 2 * C].rearrange("p (j d) -> p j d", j=H)

                p1 = prodp.tile([128, H, H, D], bf16, tag="p1")
                nc.vector.tensor_mul(
                    p1,
                    q[:, :, None, :].to_broadcast((128, H, H, D)),
                    kk[:, None, :, :].to_broadcast((128, H, H, D)),
                )
                # Tree-reduce over d: packed 2-byte adds run in the DVE 2x
                # mode, unlike TensorReduce.
                t32 = treep.tile([128, H * H, 32], f16, tag="t32")
                nc.vector.tensor_add(t32, p1[:, :, :, 0:32], p1[:, :, :, 32:64])
                t16 = treep.tile([128, H * H, 16], f16, tag="t16")
                nc.vector.tensor_add(t16, t32[:, :, 0:16], t32[:, :, 16:32])
                t8 = treep.tile([128, H * H, 8], f16, tag="t8")
                nc.vector.tensor_add(t8, t16[:, :, 0:8], t16[:, :, 8:16])
                t4 = treep.tile([128, H * H, 4], f16, tag="t4")
                nc.vector.tensor_add(t4, t8[:, :, 0:4], t8[:, :, 4:8])
                t2 = treep.tile([128, H * H, 2], f16, tag="t2")
                nc.vector.tensor_add(t2, t4[:, :, 0:2], t4[:, :, 2:4])
                logits = smallp.tile([128, H * H], f16, tag="logits")
                nc.vector.tensor_add(logits, t2[:, :, 0], t2[:, :, 1])

                # Max-subtracted softmax: keeps exp outputs <= 1 so the fp16
                # AV tree cannot overflow.
                mx = smallp.tile([128, H], f16, tag="mx")
                nc.vector.tensor_reduce(
                    mx,
                    logits.rearrange("p (i j) -> p i j", i=H),
                    axis=AX.X,
                    op=ALU.max,
                )
                lsub = smallp.tile([128, H * H], f16, tag="lsub")
                nc.vector.tensor_sub(
                    lsub.rearrange("p (i j) -> p i j", i=H),
                    logits.rearrange("p (i j) -> p i j", i=H),
                    mx[:, :, None].to_broadcast((128, H, H)),
                )
                probs = smallp.tile([128, H * H], bf16, tag="probs")
                exp_i = nc.scalar.activation(
                    probs, lsub, ACTF.Exp, scale=1.0 / np.sqrt(D)
                )
                prev_exp[0] = exp_i
                sums = smallp.tile([128, H], f32, tag="sums")
                nc.vector.tensor_reduce(
                    sums,
                    probs.rearrange("p (i j) -> p i j", i=H),
                    axis=AX.X,
                    op=ALU.add,
                )
                recip = smallp.tile([128, H], f32, tag="recip")
                nc.vector.reciprocal(recip, sums)

                # P2[p, i, d, j] = probs[p, i, j] * vT[p, d, j]; all streams
                # packed along j.
                p2 = prodp.tile([128, H, D, H], bf16, tag="p2")
                nc.vector.tensor_mul(
                    p2,
                    probs.rearrange("p (i j) -> p i j", i=H)[
                        :, :, None, :
                    ].to_broadcast((128, H, D, H)),
                    vt.rearrange("p (d j) -> p d j", d=D)[
                        :, None, :, :
                    ].to_broadcast((128, H, D, H)),
                )
                a4 = treep.tile([128, H * D, 4], f16, tag="a4")
                nc.vector.tensor_add(a4, p2[:, :, :, 0:4], p2[:, :, :, 4:8])
                a2 = treep.tile([128, H * D, 2], f16, tag="a2")
                nc.vector.tensor_add(a2, a4[:, :, 0:2], a4[:, :, 2:4])
                avout = smallp.tile([128, C], f16, tag="avout")
                nc.vector.tensor_add(avout, a2[:, :, 0], a2[:, :, 1])

                outsb = outp.tile([128, C], bf16, tag="outsb")
                nc.vector.tensor_mul(
                    outsb.rearrange("p (i d) -> p i d", i=H),
                    avout.rearrange("p (i d) -> p i d", i=H),
                    recip[:, :, None].to_broadcast((128, H, D)),
                )

                # --- scrambled flatten to DRAM bounce buffer ---
                # y[h*512 + 16k + s, t*64 + d] = out[8s + t, h*64 + d]
                dst = yg[:, 16 * k : 16 * (k + 1), :].rearrange(
                    "h s (t d) -> s t h d", t=8
                )
                nc.gpsimd.dma_start(out=dst, in_=outsb)

                # Software pipeline: previous group's proj tile between this
                # group's token tiles, so its PSUM copy never queues 8-deep
                # in front of exp on ACT.
                if yg_prev is not None:
                    emit_proj(yg_prev, g - 1, k)

            yg_prev = yg

        # Drain: last group's proj tiles.
        for h in range(H):
            emit_proj(yg_prev, NG - 1, h)

    if split_waits:
        _split_excess_waits(nc, mybir)
    return nc


def _get_runner(with_bias=False):
    """Build the Bass module once per bias-variant; return a cached callable
    that runs the SPMD kernel on 8 cores with device-cached weights/zeros."""
    key = ("runner", with_bias)
    if key in _STATE:
        return _STATE[key]

    import jax
    import concourse.mybir as mybir
    from concourse import bass2jax
    from jax.sharding import Mesh, NamedSharding, PartitionSpec

    try:
        from jax.experimental.shard_map import shard_map
    except ImportError:
        from jax import shard_map

    nc = _build_nc(with_bias=with_bias)
    bass2jax.install_neuronx_cc_hook()

    partition_name = (
        nc.partition_id_tensor.name if nc.partition_id_tensor else None
    )

    in_names = []
    out_names = []
    out_avals = []
    zero_shapes = []
    for alloc in nc.m.functions[0].allocations:
        if not isinstance(alloc, mybir.MemoryLocationSet):
            continue
        if not alloc.memorylocations:
            continue
        name = alloc.memorylocations[0].name
        if alloc.kind == "ExternalInput":
            if name != partition_name:
                in_names.append(name)
        elif alloc.kind == "ExternalOutput":
            out_names.append(name)
            shape = tuple(alloc.tensor_shape)
            dtype = mybir.dt.np(alloc.dtype)
            out_avals.append(jax.core.ShapedArray(shape, dtype))
            zero_shapes.append((shape, dtype))

    all_in_names = in_names + out_names
    if partition_name is not None:
        all_in_names = all_in_names + [partition_name]

    def _body(*args):
        operands = list(args)
        if partition_name is not None:
            operands.append(bass2jax.partition_id_tensor())
        outs = bass2jax._bass_exec_p.bind(
            *operands,
            out_avals=tuple(out_avals),
            in_names=tuple(all_in_names),
            out_names=tuple(out_names),
            lowering_input_output_aliases=(),
            sim_require_finite=True,
            sim_require_nnan=True,
            nc=nc,
        )
        return tuple(outs)

    devices = jax.devices()[:B]
    mesh = Mesh(np.asarray(devices), ("core",))
    n_args = len(in_names) + len(zero_shapes)
    sharded = jax.jit(
        shard_map(
            _body,
            mesh=mesh,
            in_specs=(PartitionSpec("core"),) * n_args,
            out_specs=(PartitionSpec("core"),) * len(out_names),
            check_rep=False,
        ),
        keep_unused=True,
    )
    sh = NamedSharding(mesh, PartitionSpec("core"))

    # Output scratch buffers: the kernel writes every element of "out", so
    # the initial contents are irrelevant -- stage once and reuse.
    zeros_staged = [
        jax.device_put(np.zeros((B * s[0],) + tuple(s[1:]), d), sh)
        for (s, d) in zero_shapes
    ]
    staged_cache = {}

    def _stage_cached(name, arr_concat):
        h = hashlib.blake2b(arr_concat.tobytes(), digest_size=16).digest()
        hit = staged_cache.get(name)
        if hit is not None and hit[0] == h:
            return hit[1]
        dev = jax.device_put(arr_concat, sh)
        staged_cache[name] = (h, dev)
        return dev

    def run(per_core_inputs):
        # per_core_inputs: list of B dicts name->np array (per-core shapes).
        args = []
        for nm in in_names:
            cat = np.concatenate(
                [per_core_inputs[c][nm] for c in range(B)], axis=0
            )
            if nm == "xpT":  # changes every call; skip the hash/caching
                args.append(jax.device_put(cat, sh))
            else:
                args.append(_stage_cached(nm, cat))
        out_arrs = sharded(*args, *zeros_staged)
        return [np.asarray(a) for a in out_arrs]

    parts = {
        "nc": nc,
        "body": _body,
        "mesh": mesh,
        "in_specs": (PartitionSpec("core"),) * n_args,
        "out_specs": (PartitionSpec("core"),) * len(out_names),
        "in_names": in_names,
        "out_names": out_names,
        "zero_shapes": zero_shapes,
    }
    _STATE["parts"] = parts
    _STATE[key] = run
    return run


def kernel(x, pos_32, w_qkv, b_qkv, w_proj, b_proj, resolution):
    import ml_dtypes

    bf = ml_dtypes.bfloat16

    x = np.asarray(x, dtype=np.float32)
    pos_32 = np.asarray(pos_32, dtype=np.float32)
    w_qkv = np.asarray(w_qkv, dtype=np.float32)
    b_qkv = np.asarray(b_qkv, dtype=np.float32)
    w_proj = np.asarray(w_proj, dtype=np.float32)
    b_proj = np.asarray(b_proj, dtype=np.float32)

    Bx, Nx, Cx = x.shape
    target_len = int(resolution) ** 3
    pos = _interp_linear_np(pos_32, target_len)
    xp = x + pos if pos.shape[1] == Nx else x

    # [B, C, N] bf16: transposed so lhsT tiles load contiguously.
    xpT = np.ascontiguousarray(np.transpose(xp, (0, 2, 1))).astype(bf)

    with_bias = bool(b_qkv.any() or b_proj.any())
    run = _get_runner(with_bias=with_bias)

    wq = w_qkv.astype(bf)
    bq = b_qkv.reshape(1, 3 * C).astype(bf)
    wpr = w_proj.astype(bf)
    bp = b_proj.reshape(1, C).astype(bf)
    per_core = [
        {"xpT": xpT[b], "wqkv": wq, "bqkv": bq, "wproj": wpr, "bproj": bp}
        for b in range(Bx)
    ]
    outs = run(per_core)
    return outs[0].reshape(Bx, Nx, Cx).astype(np.float32)


# revision 28
# speedup vs baseline: 1.0689x; 1.0689x over previous
"""AdaptiveAttention kernel for 8 trn2 NeuronCores — Bass/Tile implementation.

Strategy (per sharding hint): data-parallel over batch B=8, one batch element
per NeuronCore; weights replicated. Host-side preprocessing (cheap, per-token
independent): interpolate the pos table for the given resolution, add it to x,
and store x+pos transposed as bf16 so the device loads matmul lhsT tiles
directly. Device per core:

  1. qkv = (x+pos) @ w_qkv + b_qkv    -- PE matmuls (bf16)
  2. per-token 8x8 head attention     -- DVE broadcast-product + packed-2-byte
                                         tree reduction (TensorReduce has no
                                         2x mode, packed adds do), exp on ACT
  3. scrambled flatten ([N,H,D] -> transpose(1,2).reshape(N,C)) -- folded
     into a strided DMA through a DRAM bounce buffer
  4. out = y @ w_proj + b_proj        -- XBAR transpose-load of y, PE matmuls

Hardcoded problem shape: B=8, N=4096, C=512, H=8, D=64.
"""

import hashlib
import os
import sys

import numpy as np

# Persistent XLA/NEFF compilation cache: a fresh process reuses the compiled
# executable instead of re-running the Neuron compiler (~8 s).
os.environ.setdefault("JAX_COMPILATION_CACHE_DIR", "/tmp/jax_comp_cache")
os.environ.setdefault("JAX_PERSISTENT_CACHE_MIN_COMPILE_TIME_SECS", "1")

for _p in ("/opt/trn_rl_repo",):
    if _p not in sys.path:
        sys.path.append(_p)

B, N, C = 8, 4096, 512
H = 8
D = C // H  # 64
NG = 4          # token groups per core
GTOK = N // NG  # 1024 tokens per group
TT = GTOK // 128  # 8 token-tiles per group

_STATE = {}


def _interp_linear_np(pos, out_len):
    # F.interpolate(mode='linear', align_corners=False) along axis 1.
    in_len = pos.shape[1]
    if in_len == out_len:
        return pos
    scale = in_len / out_len
    coords = (np.arange(out_len, dtype=np.float64) + 0.5) * scale - 0.5
    coords = np.clip(coords, 0.0, in_len - 1)
    i0 = np.floor(coords).astype(np.int64)
    i1 = np.minimum(i0 + 1, in_len - 1)
    w = (coords - i0).astype(np.float32)[None, :, None]
    return pos[:, i0, :] * (1.0 - w) + pos[:, i1, :] * w


def _split_excess_waits(nc, mybir):
    """Walrus codegen in this container accepts at most one sync-wait per
    instruction; hoist extra waits into standalone EventSemaphore
    instructions on the same engine queue (executed in order, so semantics
    are preserved)."""
    n_split = 0
    for fn in nc.m.functions:
        for blk in fn.blocks:
            insts = blk.instructions
            out = []
            changed = False
            for inst in insts:
                si = inst.sync_info
                if si is not None and len(si.on_wait) > 1:
                    for i, w in enumerate(si.on_wait[:-1]):
                        ev = mybir.InstEventSemaphore(
                            name=f"{inst.name}-wsplit{i}"
                        )
                        ev.engine = inst.engine
                        ev.sync_info = mybir.SyncInfo(on_wait=[w], on_update=[])
                        out.append(ev)
                    inst.sync_info = mybir.SyncInfo(
                        on_wait=[si.on_wait[-1]], on_update=list(si.on_update)
                    )
                    out.append(inst)
                    changed = True
                    n_split += 1
                else:
                    out.append(inst)
            if changed:
                blk.instructions = out
    return n_split


def _build_nc(split_waits=True, with_bias=False):
    from contextlib import ExitStack

    import concourse.bass as bass
    import concourse.mybir as mybir
    import concourse.tile as tile
    from concourse.tile import add_dep_helper

    dt = mybir.dt
    bf16 = dt.bfloat16
    f16 = dt.float16
    f32 = dt.float32
    AX = mybir.AxisListType
    ALU = mybir.AluOpType
    ACTF = mybir.ActivationFunctionType

    nc = bass.Bass(num_swdge_queues=4)
    xpT = nc.declare_dram_parameter("xpT", [C, N], bf16, isOutput=False)
    wqkv = nc.declare_dram_parameter("wqkv", [C, 3 * C], bf16, isOutput=False)
    bqkv = nc.declare_dram_parameter("bqkv", [1, 3 * C], bf16, isOutput=False)
    wproj = nc.declare_dram_parameter("wproj", [C, C], bf16, isOutput=False)
    bproj = nc.declare_dram_parameter("bproj", [1, C], bf16, isOutput=False)
    out = nc.declare_dram_parameter("out", [N, C], bf16, isOutput=True)

    with tile.TileContext(nc) as tc, ExitStack() as ctx:
        const = ctx.enter_context(tc.tile_pool(name="const", bufs=1))
        xptp = ctx.enter_context(tc.tile_pool(name="xptp", bufs=2))
        qkvp = ctx.enter_context(tc.tile_pool(name="qkvp", bufs=3))
        prodp = ctx.enter_context(tc.tile_pool(name="prodp", bufs=2))
        treep = ctx.enter_context(tc.tile_pool(name="treep", bufs=2))
        smallp = ctx.enter_context(tc.tile_pool(name="smallp", bufs=3))
        outp = ctx.enter_context(tc.tile_pool(name="outp", bufs=3))
        ytp = ctx.enter_context(tc.tile_pool(name="ytp", bufs=8))
        psump = ctx.enter_context(tc.tile_pool(name="psump", bufs=2, space="PSUM"))
        dramp = ctx.enter_context(tc.tile_pool(name="dramp", bufs=2, space="DRAM"))

        # Replicated weights in SBUF. [128, cc, f] with cc = contraction chunk.
        wq = const.tile([128, 4, 3 * C], bf16)
        nc.scalar.dma_start(out=wq, in_=wqkv.rearrange("(cc p) f -> p cc f", p=128))
        wp = const.tile([128, 4, C], bf16)
        nc.scalar.dma_start(out=wp, in_=wproj.rearrange("(cc p) f -> p cc f", p=128))
        if with_bias:
            # Biases broadcast to all partitions (DVE cannot partition-bcast).
            bq = const.tile([128, 3 * C], bf16)
            nc.scalar.dma_start(out=bq, in_=bqkv[:].to_broadcast((128, 3 * C)))
            bp = const.tile([128, C], bf16)
            nc.scalar.dma_start(out=bp, in_=bproj[:].to_broadcast((128, C)))

        def emit_proj(yg_s, g_s, h):
            # One output row-tile: rows h*512+128*g_s .. +128.
            yt = ytp.tile([128, 4, 128], bf16, tag="yt")
            # One blocked XBAR transpose: [128, 512] -> [512, 128] delivered
            # as 4 partition blocks stacked on the free axis.
            nc.sync.dma_start_transpose(yt, yg_s[h])
            pso = psump.tile([128, C], f32, tag="proj_ps")
            for cc in range(4):
                nc.tensor.matmul(
                    pso,
                    lhsT=yt[:, cc, :],
                    rhs=wp[:, cc, :],
                    start=(cc == 0),
                    stop=(cc == 3),
                )
            osb = outp.tile([128, C], bf16, tag="osb")
            if with_bias:
                nc.vector.tensor_add(osb, pso, bp)
            else:
                osb_i = nc.scalar.activation(osb, pso, ACTF.Copy)
                if prev_exp[0] is not None:
                    add_dep_helper(
                        osb_i.ins, prev_exp[0].ins, sync=False,
                        reason="exp before proj copy on ACT",
                    )
            nc.gpsimd.dma_start(
                out=out[h * 512 + 128 * g_s : h * 512 + 128 * (g_s + 1), :],
                in_=osb,
            )

        yg_prev = None
        prev_exp = [None]
        for g in range(NG):
            # x+pos transposed slab for this group: [c-part, cc, token]
            xpt = xptp.tile([128, 4, GTOK], bf16, tag="xpt")
            nc.gpsimd.dma_start(
                out=xpt,
                in_=xpT.rearrange("(cc p) n -> p cc n", p=128)[
                    :, :, g * GTOK : (g + 1) * GTOK
                ],
            )
            # Scrambled attention output bounce buffer (y rows for this group).
            yg = dramp.tile([H, 128, C], bf16, tag="yg")

            for k in range(TT // 2):
                # Two 128-token subtiles share each DVE instruction (pair
                # dim on the free axis) to halve per-op fixed overheads.
                qk2 = qkvp.tile([128, 2, 2 * C], bf16, tag="qk")
                vt2 = qkvp.tile([128, 2, C], bf16, tag="vt")
                for r in range(2):
                    k2 = 2 * k + r
                    ps = psump.tile([128, 3 * C], f32, tag="qkv_ps")
                    for oc in range(3):
                        for cc in range(4):
                            nc.tensor.matmul(
                                ps[:, oc * 512 : (oc + 1) * 512],
                                lhsT=xpt[:, cc, k2 * 128 : (k2 + 1) * 128],
                                rhs=wq[:, cc, oc * 512 : (oc + 1) * 512],
                                start=(cc == 0),
                                stop=(cc == 3),
                            )
                    vt_jd = vt2[:, r, :].rearrange("p (d j) -> p j d", d=D)
                    vsrc = ps[:, 2 * C : 3 * C].rearrange("p (j d) -> p j d", j=H)
                    if with_bias:
                        nc.vector.tensor_add(
                            qk2[:, r, :], ps[:, 0 : 2 * C], bq[:, 0 : 2 * C]
                        )
                        nc.vector.tensor_add(
                            vt_jd,
                            vsrc,
                            bq[:, 2 * C : 3 * C].rearrange(
                                "p (j d) -> p j d", j=H
                            ),
                        )
                    else:
                        qk_i = nc.scalar.activation(
                            qk2[:, r, :], ps[:, 0 : 2 * C], ACTF.Copy
                        )
                        nc.scalar.activation(vt_jd, vsrc, ACTF.Copy)
                        if r == 0 and prev_exp[0] is not None:
                            # Keep the previous pair's exp ahead of these
                            # PE-gated copies in the in-order ACT queue.
                            add_dep_helper(
                                qk_i.ins, prev_exp[0].ins, sync=False,
                                reason="exp before next-pair ACT copies",
                            )

                # --- per-token 8x8 head attention (paired) ---
                q = qk2[:, :, 0:C].rearrange("p r (i d) -> p r i d", i=H)
                kk = qk2[:, :, C : 2 * C].rearrange("p r (j d) -> p r j d", j=H)

                p1 = prodp.tile([128, 2, H, H, D], bf16, tag="p1")
                for r in range(2):
                    nc.vector.tensor_mul(
                        p1[:, r],
                        q[:, r, :, None, :].to_broadcast((128, H, H, D)),
                        kk[:, r, None, :, :].to_broadcast((128, H, H, D)),
                    )
                p1f = p1.rearrange("p r i j d -> p r (i j) d")
                t32 = treep.tile([128, 2, H * H, 32], f16, tag="t32")
                nc.vector.tensor_add(t32, p1f[:, :, :, 0:32], p1f[:, :, :, 32:64])
                t16 = treep.tile([128, 2, H * H, 16], f16, tag="t16")
                nc.vector.tensor_add(t16, t32[:, :, :, 0:16], t32[:, :, :, 16:32])
                t8 = treep.tile([128, 2, H * H, 8], f16, tag="t8")
                nc.vector.tensor_add(t8, t16[:, :, :, 0:8], t16[:, :, :, 8:16])
                t4 = treep.tile([128, 2, H * H, 4], f16, tag="t4")
                nc.vector.tensor_add(t4, t8[:, :, :, 0:4], t8[:, :, :, 4:8])
                t2 = treep.tile([128, 2, H * H, 2], f16, tag="t2")
                nc.vector.tensor_add(t2, t4[:, :, :, 0:2], t4[:, :, :, 2:4])
                logits = smallp.tile([128, 2, H * H], f16, tag="logits")
                nc.vector.tensor_add(logits, t2[:, :, :, 0], t2[:, :, :, 1])

                # Max-subtracted softmax: keeps exp outputs <= 1 so the fp16
                # AV tree cannot overflow.
                mx = smallp.tile([128, 2, H], f16, tag="mx")
                nc.vector.tensor_reduce(
                    mx,
                    logits.rearrange("p r (i j) -> p r i j", i=H),
                    axis=AX.X,
                    op=ALU.max,
                )
                lsub = smallp.tile([128, 2, H * H], f16, tag="lsub")
                nc.vector.tensor_sub(
                    lsub.rearrange("p r (i j) -> p r i j", i=H),
                    logits.rearrange("p r (i j) -> p r i j", i=H),
                    mx[:, :, :, None].to_broadcast((128, 2, H, H)),
                )
                probs = smallp.tile([128, 2, H * H], bf16, tag="probs")
                exp_i = nc.scalar.activation(
                    probs, lsub, ACTF.Exp, scale=1.0 / np.sqrt(D)
                )
                prev_exp[0] = exp_i
                sums = smallp.tile([128, 2, H], f32, tag="sums")
                nc.vector.tensor_reduce(
                    sums,
                    probs.rearrange("p r (i j) -> p r i j", i=H),
                    axis=AX.X,
                    op=ALU.add,
                )
                recip = smallp.tile([128, 2, H], f32, tag="recip")
                nc.vector.reciprocal(recip, sums)

                p2 = prodp.tile([128, 2, H, D, H], bf16, tag="p2")
                for r in range(2):
                    nc.vector.tensor_mul(
                        p2[:, r],
                        probs.rearrange("p r (i j) -> p r i j", i=H)[
                            :, r, :, None, :
                        ].to_broadcast((128, H, D, H)),
                        vt2[:, r, :].rearrange("p (d j) -> p d j", d=D)[
                            :, None, :, :
                        ].to_broadcast((128, H, D, H)),
                    )
                p2f = p2.rearrange("p r i d j -> p r (i d) j")
                a4 = treep.tile([128, 2, H * D, 4], f16, tag="a4")
                nc.vector.tensor_add(a4, p2f[:, :, :, 0:4], p2f[:, :, :, 4:8])
                a2 = treep.tile([128, 2, H * D, 2], f16, tag="a2")
                nc.vector.tensor_add(a2, a4[:, :, :, 0:2], a4[:, :, :, 2:4])
                avout = smallp.tile([128, 2, C], f16, tag="avout")
                nc.vector.tensor_add(avout, a2[:, :, :, 0], a2[:, :, :, 1])

                outsb = outp.tile([128, 2, C], bf16, tag="outsb")
                nc.vector.tensor_mul(
                    outsb.rearrange("p r (i d) -> p r i d", i=H),
                    avout.rearrange("p r (i d) -> p r i d", i=H),
                    recip[:, :, :, None].to_broadcast((128, 2, H, D)),
                )

                # --- scrambled flatten to DRAM bounce buffer ---
                # y[h*512 + 16k + s, t*64 + d] = out[8s + t, h*64 + d]
                for r in range(2):
                    k2 = 2 * k + r
                    dst = yg[:, 16 * k2 : 16 * (k2 + 1), :].rearrange(
                        "h s (t d) -> s t h d", t=8
                    )
                    nc.gpsimd.dma_start(out=dst, in_=outsb[:, r, :])

                # Software pipeline: previous group's proj tiles between
                # this group's token-tile pairs, so their PSUM copies never
                # queue 8-deep in front of exp on ACT.
                if yg_prev is not None:
                    emit_proj(yg_prev, g - 1, 2 * k)
                    emit_proj(yg_prev, g - 1, 2 * k + 1)

            yg_prev = yg

        # Drain: last group's proj tiles.
        for h in range(H):
            emit_proj(yg_prev, NG - 1, h)

    if split_waits:
        _split_excess_waits(nc, mybir)
    return nc


def _get_runner(with_bias=False):
    """Build the Bass module once per bias-variant; return a cached callable
    that runs the SPMD kernel on 8 cores with device-cached weights/zeros."""
    key = ("runner", with_bias)
    if key in _STATE:
        return _STATE[key]

    import jax
    import concourse.mybir as mybir
    from concourse import bass2jax
    from jax.sharding import Mesh, NamedSharding, PartitionSpec

    try:
        from jax.experimental.shard_map import shard_map
    except ImportError:
        from jax import shard_map

    nc = _build_nc(with_bias=with_bias)
    bass2jax.install_neuronx_cc_hook()

    partition_name = (
        nc.partition_id_tensor.name if nc.partition_id_tensor else None
    )

    in_names = []
    out_names = []
    out_avals = []
    zero_shapes = []
    for alloc in nc.m.functions[0].allocations:
        if not isinstance(alloc, mybir.MemoryLocationSet):
            continue
        if not alloc.memorylocations:
            continue
        name = alloc.memorylocations[0].name
        if alloc.kind == "ExternalInput":
            if name != partition_name:
                in_names.append(name)
        elif alloc.kind == "ExternalOutput":
            out_names.append(name)
            shape = tuple(alloc.tensor_shape)
            dtype = mybir.dt.np(alloc.dtype)
            out_avals.append(jax.core.ShapedArray(shape, dtype))
            zero_shapes.append((shape, dtype))

    all_in_names = in_names + out_names
    if partition_name is not None:
        all_in_names = all_in_names + [partition_name]

    def _body(*args):
        operands = list(args)
        if partition_name is not None:
            operands.append(bass2jax.partition_id_tensor())
        outs = bass2jax._bass_exec_p.bind(
            *operands,
            out_avals=tuple(out_avals),
            in_names=tuple(all_in_names),
            out_names=tuple(out_names),
            lowering_input_output_aliases=(),
            sim_require_finite=True,
            sim_require_nnan=True,
            nc=nc,
        )
        return tuple(outs)

    devices = jax.devices()[:B]
    mesh = Mesh(np.asarray(devices), ("core",))
    n_args = len(in_names) + len(zero_shapes)
    sharded = jax.jit(
        shard_map(
            _body,
            mesh=mesh,
            in_specs=(PartitionSpec("core"),) * n_args,
            out_specs=(PartitionSpec("core"),) * len(out_names),
            check_rep=False,
        ),
        keep_unused=True,
    )
    sh = NamedSharding(mesh, PartitionSpec("core"))

    # Output scratch buffers: the kernel writes every element of "out", so
    # the initial contents are irrelevant -- stage once and reuse.
    zeros_staged = [
        jax.device_put(np.zeros((B * s[0],) + tuple(s[1:]), d), sh)
        for (s, d) in zero_shapes
    ]
    staged_cache = {}

    def _stage_cached(name, arr_concat):
        h = hashlib.blake2b(arr_concat.tobytes(), digest_size=16).digest()
        hit = staged_cache.get(name)
        if hit is not None and hit[0] == h:
            return hit[1]
        dev = jax.device_put(arr_concat, sh)
        staged_cache[name] = (h, dev)
        return dev

    def run(per_core_inputs):
        # per_core_inputs: list of B dicts name->np array (per-core shapes).
        args = []
        for nm in in_names:
            cat = np.concatenate(
                [per_core_inputs[c][nm] for c in range(B)], axis=0
            )
            if nm == "xpT":  # changes every call; skip the hash/caching
                args.append(jax.device_put(cat, sh))
            else:
                args.append(_stage_cached(nm, cat))
        out_arrs = sharded(*args, *zeros_staged)
        return [np.asarray(a) for a in out_arrs]

    parts = {
        "nc": nc,
        "body": _body,
        "mesh": mesh,
        "in_specs": (PartitionSpec("core"),) * n_args,
        "out_specs": (PartitionSpec("core"),) * len(out_names),
        "in_names": in_names,
        "out_names": out_names,
        "zero_shapes": zero_shapes,
    }
    _STATE["parts"] = parts
    _STATE[key] = run
    return run


def kernel(x, pos_32, w_qkv, b_qkv, w_proj, b_proj, resolution):
    import ml_dtypes

    bf = ml_dtypes.bfloat16

    x = np.asarray(x, dtype=np.float32)
    pos_32 = np.asarray(pos_32, dtype=np.float32)
    w_qkv = np.asarray(w_qkv, dtype=np.float32)
    b_qkv = np.asarray(b_qkv, dtype=np.float32)
    w_proj = np.asarray(w_proj, dtype=np.float32)
    b_proj = np.asarray(b_proj, dtype=np.float32)

    Bx, Nx, Cx = x.shape
    target_len = int(resolution) ** 3
    pos = _interp_linear_np(pos_32, target_len)
    xp = x + pos if pos.shape[1] == Nx else x

    # [B, C, N] bf16: transposed so lhsT tiles load contiguously.
    xpT = np.ascontiguousarray(np.transpose(xp, (0, 2, 1))).astype(bf)

    with_bias = bool(b_qkv.any() or b_proj.any())
    run = _get_runner(with_bias=with_bias)

    wq = w_qkv.astype(bf)
    bq = b_qkv.reshape(1, 3 * C).astype(bf)
    wpr = w_proj.astype(bf)
    bp = b_proj.reshape(1, C).astype(bf)
    per_core = [
        {"xpT": xpT[b], "wqkv": wq, "bqkv": bq, "wproj": wpr, "bproj": bp}
        for b in range(Bx)
    ]
    outs = run(per_core)
    return outs[0].reshape(Bx, Nx, Cx).astype(np.float32)


# revision 33
# speedup vs baseline: 1.0824x; 1.0126x over previous
"""AdaptiveAttention kernel for 8 trn2 NeuronCores — Bass/Tile implementation.

Strategy (per sharding hint): data-parallel over batch B=8, one batch element
per NeuronCore; weights replicated. Host-side preprocessing (cheap, per-token
independent): interpolate the pos table for the given resolution, add it to x,
and store x+pos transposed as bf16 so the device loads matmul lhsT tiles
directly. Device per core:

  1. qkv = (x+pos) @ w_qkv + b_qkv    -- PE matmuls (bf16)
  2. per-token 8x8 head attention     -- DVE broadcast-product + packed-2-byte
                                         tree reduction (TensorReduce has no
                                         2x mode, packed adds do), exp on ACT
  3. scrambled flatten ([N,H,D] -> transpose(1,2).reshape(N,C)) -- folded
     into a strided DMA through a DRAM bounce buffer
  4. out = y @ w_proj + b_proj        -- XBAR transpose-load of y, PE matmuls

Hardcoded problem shape: B=8, N=4096, C=512, H=8, D=64.
"""

import hashlib
import os
import sys

import numpy as np

# Persistent XLA/NEFF compilation cache: a fresh process reuses the compiled
# executable instead of re-running the Neuron compiler (~8 s).
os.environ.setdefault("JAX_COMPILATION_CACHE_DIR", "/tmp/jax_comp_cache")
os.environ.setdefault("JAX_PERSISTENT_CACHE_MIN_COMPILE_TIME_SECS", "1")

for _p in ("/opt/trn_rl_repo",):
    if _p not in sys.path:
        sys.path.append(_p)

B, N, C = 8, 4096, 512
H = 8
D = C // H  # 64
NG = 4          # token groups per core
GTOK = N // NG  # 1024 tokens per group
TT = GTOK // 128  # 8 token-tiles per group

_STATE = {}


def _interp_linear_np(pos, out_len):
    # F.interpolate(mode='linear', align_corners=False) along axis 1.
    in_len = pos.shape[1]
    if in_len == out_len:
        return pos
    scale = in_len / out_len
    coords = (np.arange(out_len, dtype=np.float64) + 0.5) * scale - 0.5
    coords = np.clip(coords, 0.0, in_len - 1)
    i0 = np.floor(coords).astype(np.int64)
    i1 = np.minimum(i0 + 1, in_len - 1)
    w = (coords - i0).astype(np.float32)[None, :, None]
    return pos[:, i0, :] * (1.0 - w) + pos[:, i1, :] * w


def _split_excess_waits(nc, mybir):
    """Walrus codegen in this container accepts at most one sync-wait per
    instruction; hoist extra waits into standalone EventSemaphore
    instructions on the same engine queue (executed in order, so semantics
    are preserved)."""
    n_split = 0
    for fn in nc.m.functions:
        for blk in fn.blocks:
            insts = blk.instructions
            out = []
            changed = False
            for inst in insts:
                si = inst.sync_info
                if si is not None and len(si.on_wait) > 1:
                    for i, w in enumerate(si.on_wait[:-1]):
                        ev = mybir.InstEventSemaphore(
                            name=f"{inst.name}-wsplit{i}"
                        )
                        ev.engine = inst.engine
                        ev.sync_info = mybir.SyncInfo(on_wait=[w], on_update=[])
                        out.append(ev)
                    inst.sync_info = mybir.SyncInfo(
                        on_wait=[si.on_wait[-1]], on_update=list(si.on_update)
                    )
                    out.append(inst)
                    changed = True
                    n_split += 1
                else:
                    out.append(inst)
            if changed:
                blk.instructions = out
    return n_split


def _build_nc(split_waits=True, with_bias=False):
    from contextlib import ExitStack

    import concourse.bass as bass
    import concourse.mybir as mybir
    import concourse.tile as tile
    from concourse.tile import add_dep_helper

    dt = mybir.dt
    bf16 = dt.bfloat16
    f16 = dt.float16
    f32 = dt.float32
    AX = mybir.AxisListType
    ALU = mybir.AluOpType
    ACTF = mybir.ActivationFunctionType

    nc = bass.Bass(num_swdge_queues=4)
    xpT = nc.declare_dram_parameter("xpT", [C, N], bf16, isOutput=False)
    wqkv = nc.declare_dram_parameter("wqkv", [C, 3 * C], bf16, isOutput=False)
    bqkv = nc.declare_dram_parameter("bqkv", [1, 3 * C], bf16, isOutput=False)
    wproj = nc.declare_dram_parameter("wproj", [C, C], bf16, isOutput=False)
    bproj = nc.declare_dram_parameter("bproj", [1, C], bf16, isOutput=False)
    out = nc.declare_dram_parameter("out", [N, C], bf16, isOutput=True)

    with tile.TileContext(nc) as tc, ExitStack() as ctx:
        const = ctx.enter_context(tc.tile_pool(name="const", bufs=1))
        xptp = ctx.enter_context(tc.tile_pool(name="xptp", bufs=2))
        qkvp = ctx.enter_context(tc.tile_pool(name="qkvp", bufs=3))
        prodp = ctx.enter_context(tc.tile_pool(name="prodp", bufs=2))
        treep = ctx.enter_context(tc.tile_pool(name="treep", bufs=2))
        smallp = ctx.enter_context(tc.tile_pool(name="smallp", bufs=3))
        outp = ctx.enter_context(tc.tile_pool(name="outp", bufs=3))
        ytp = ctx.enter_context(tc.tile_pool(name="ytp", bufs=8))
        psump = ctx.enter_context(tc.tile_pool(name="psump", bufs=2, space="PSUM"))
        dramp = ctx.enter_context(tc.tile_pool(name="dramp", bufs=2, space="DRAM"))

        # Replicated weights in SBUF. [128, cc, f] with cc = contraction chunk.
        wq3 = []
        for oc in range(3):
            wqo = const.tile([128, 4, C], bf16, tag=f"wq{oc}")
            nc.scalar.dma_start(
                out=wqo,
                in_=wqkv[:, oc * 512 : (oc + 1) * 512].rearrange(
                    "(cc p) f -> p cc f", p=128
                ),
            )
            wq3.append(wqo)
        wp = const.tile([128, 4, C], bf16)
        nc.scalar.dma_start(out=wp, in_=wproj.rearrange("(cc p) f -> p cc f", p=128))
        if with_bias:
            # Biases broadcast to all partitions (DVE cannot partition-bcast).
            bq = const.tile([128, 3 * C], bf16)
            nc.scalar.dma_start(out=bq, in_=bqkv[:].to_broadcast((128, 3 * C)))
            bp = const.tile([128, C], bf16)
            nc.scalar.dma_start(out=bp, in_=bproj[:].to_broadcast((128, C)))

        def emit_proj(yg_s, g_s, h):
            # One output row-tile: rows h*512+128*g_s .. +128.
            yt = ytp.tile([128, 4, 128], bf16, tag="yt")
            # One blocked XBAR transpose: [128, 512] -> [512, 128] delivered
            # as 4 partition blocks stacked on the free axis.
            nc.sync.dma_start_transpose(yt, yg_s[h])
            pso = psump.tile([128, C], f32, tag="proj_ps")
            for cc in range(4):
                nc.tensor.matmul(
                    pso,
                    lhsT=yt[:, cc, :],
                    rhs=wp[:, cc, :],
                    start=(cc == 0),
                    stop=(cc == 3),
                )
            osb = outp.tile([128, C], bf16, tag="osb")
            if with_bias:
                nc.vector.tensor_add(osb, pso, bp)
            else:
                osb_i = nc.scalar.activation(osb, pso, ACTF.Copy)
                if prev_exp[0] is not None:
                    add_dep_helper(
                        osb_i.ins, prev_exp[0].ins, sync=False,
                        reason="exp before proj copy on ACT",
                    )
            nc.gpsimd.dma_start(
                out=out[h * 512 + 128 * g_s : h * 512 + 128 * (g_s + 1), :],
                in_=osb,
            )

        yg_prev = None
        prev_exp = [None]
        for g in range(NG):
            # x+pos transposed slab for this group, in two half-slabs so
            # the first token-pair's matmuls don't wait for the full 2 MB.
            xpt_h = []
            for hh in range(2):
                xh = xptp.tile([128, 4, GTOK // 2], bf16, tag=f"xpt{hh}")
                nc.gpsimd.dma_start(
                    out=xh,
                    in_=xpT.rearrange("(cc p) n -> p cc n", p=128)[
                        :,
                        :,
                        g * GTOK + hh * (GTOK // 2) : g * GTOK
                        + (hh + 1) * (GTOK // 2),
                    ],
                )
                xpt_h.append(xh)
            # Scrambled attention output bounce buffer (y rows for this group).
            yg = dramp.tile([H, 128, C], bf16, tag="yg")

            for k in range(TT // 2):
                # Two 128-token subtiles share each DVE instruction (pair
                # dim on the free axis) to halve per-op fixed overheads.
                qk2 = qkvp.tile([128, 2, 2 * C], bf16, tag="qk")
                vt2 = qkvp.tile([128, 2, C], bf16, tag="vt")
                for r in range(2):
                    k2 = 2 * k + r
                    ps = psump.tile([128, 3 * C], f32, tag="qkv_ps")
                    for oc in range(3):
                        for cc in range(4):
                            nc.tensor.matmul(
                                ps[:, oc * 512 : (oc + 1) * 512],
                                lhsT=xpt_h[k2 // 4][:, cc, (k2 % 4) * 128 : (k2 % 4 + 1) * 128],
                                rhs=wq3[oc][:, cc, :],
                                start=(cc == 0),
                                stop=(cc == 3),
                            )
                    vt_jd = vt2[:, r, :].rearrange("p (d j) -> p j d", d=D)
                    vsrc = ps[:, 2 * C : 3 * C].rearrange("p (j d) -> p j d", j=H)
                    if with_bias:
                        nc.vector.tensor_add(
                            qk2[:, r, :], ps[:, 0 : 2 * C], bq[:, 0 : 2 * C]
                        )
                        nc.vector.tensor_add(
                            vt_jd,
                            vsrc,
                            bq[:, 2 * C : 3 * C].rearrange(
                                "p (j d) -> p j d", j=H
                            ),
                        )
                    else:
                        qk_i = nc.scalar.activation(
                            qk2[:, r, :], ps[:, 0 : 2 * C], ACTF.Copy
                        )
                        nc.scalar.activation(vt_jd, vsrc, ACTF.Copy)
                        if r == 0 and prev_exp[0] is not None:
                            # Keep the previous pair's exp ahead of these
                            # PE-gated copies in the in-order ACT queue.
                            add_dep_helper(
                                qk_i.ins, prev_exp[0].ins, sync=False,
                                reason="exp before next-pair ACT copies",
                            )

                # --- per-token 8x8 head attention (paired) ---
                q = qk2[:, :, 0:C].rearrange("p r (i d) -> p r i d", i=H)
                kk = qk2[:, :, C : 2 * C].rearrange("p r (j d) -> p r j d", j=H)

                p1 = prodp.tile([128, 2, H, H, D], bf16, tag="p1")
                for r in range(2):
                    nc.vector.tensor_mul(
                        p1[:, r],
                        q[:, r, :, None, :].to_broadcast((128, H, H, D)),
                        kk[:, r, None, :, :].to_broadcast((128, H, H, D)),
                    )
                p1f = p1.rearrange("p r i j d -> p r (i j) d")
                t32 = treep.tile([128, 2, H * H, 32], f16, tag="t32")
                nc.vector.tensor_add(t32, p1f[:, :, :, 0:32], p1f[:, :, :, 32:64])
                t16 = treep.tile([128, 2, H * H, 16], f16, tag="t16")
                nc.vector.tensor_add(t16, t32[:, :, :, 0:16], t32[:, :, :, 16:32])
                t8 = treep.tile([128, 2, H * H, 8], f16, tag="t8")
                nc.vector.tensor_add(t8, t16[:, :, :, 0:8], t16[:, :, :, 8:16])
                t4 = treep.tile([128, 2, H * H, 4], f16, tag="t4")
                nc.vector.tensor_add(t4, t8[:, :, :, 0:4], t8[:, :, :, 4:8])
                t2 = treep.tile([128, 2, H * H, 2], f16, tag="t2")
                nc.vector.tensor_add(t2, t4[:, :, :, 0:2], t4[:, :, :, 2:4])
                logits = smallp.tile([128, 2, H * H], f16, tag="logits")
                nc.vector.tensor_add(logits, t2[:, :, :, 0], t2[:, :, :, 1])

                # Max-subtracted softmax: keeps exp outputs <= 1 so the fp16
                # AV tree cannot overflow.
                mx = smallp.tile([128, 2, H], f16, tag="mx")
                nc.vector.tensor_reduce(
                    mx,
                    logits.rearrange("p r (i j) -> p r i j", i=H),
                    axis=AX.X,
                    op=ALU.max,
                )
                lsub = smallp.tile([128, 2, H * H], f16, tag="lsub")
                nc.vector.tensor_sub(
                    lsub.rearrange("p r (i j) -> p r i j", i=H),
                    logits.rearrange("p r (i j) -> p r i j", i=H),
                    mx[:, :, :, None].to_broadcast((128, 2, H, H)),
                )
                probs = smallp.tile([128, 2, H * H], bf16, tag="probs")
                exp_i = nc.scalar.activation(
                    probs, lsub, ACTF.Exp, scale=1.0 / np.sqrt(D)
                )
                prev_exp[0] = exp_i
                sums = smallp.tile([128, 2, H], f32, tag="sums")
                nc.vector.tensor_reduce(
                    sums,
                    probs.rearrange("p r (i j) -> p r i j", i=H),
                    axis=AX.X,
                    op=ALU.add,
                )
                recip = smallp.tile([128, 2, H], f32, tag="recip")
                nc.vector.reciprocal(recip, sums)
                pn = smallp.tile([128, 2, H * H], bf16, tag="pn")
                nc.vector.tensor_mul(
                    pn.rearrange("p r (i j) -> p r i j", i=H),
                    probs.rearrange("p r (i j) -> p r i j", i=H),
                    recip[:, :, :, None].to_broadcast((128, 2, H, H)),
                )

                p2 = prodp.tile([128, 2, H, D, H], bf16, tag="p2")
                for r in range(2):
                    nc.vector.tensor_mul(
                        p2[:, r],
                        pn.rearrange("p r (i j) -> p r i j", i=H)[
                            :, r, :, None, :
                        ].to_broadcast((128, H, D, H)),
                        vt2[:, r, :].rearrange("p (d j) -> p d j", d=D)[
                            :, None, :, :
                        ].to_broadcast((128, H, D, H)),
                    )
                p2f = p2.rearrange("p r i d j -> p r (i d) j")
                a4 = treep.tile([128, 2, H * D, 4], f16, tag="a4")
                nc.vector.tensor_add(a4, p2f[:, :, :, 0:4], p2f[:, :, :, 4:8])
                a2 = treep.tile([128, 2, H * D, 2], f16, tag="a2")
                nc.vector.tensor_add(a2, a4[:, :, :, 0:2], a4[:, :, :, 2:4])
                outsb = outp.tile([128, 2, C], bf16, tag="outsb")
                nc.vector.tensor_add(outsb, a2[:, :, :, 0], a2[:, :, :, 1])

                # --- scrambled flatten to DRAM bounce buffer ---
                # y[h*512 + 16k + s, t*64 + d] = out[8s + t, h*64 + d]
                for r in range(2):
                    k2 = 2 * k + r
                    dst = yg[:, 16 * k2 : 16 * (k2 + 1), :].rearrange(
                        "h s (t d) -> s t h d", t=8
                    )
                    nc.gpsimd.dma_start(out=dst, in_=outsb[:, r, :])

                # Software pipeline: previous group's proj tiles between
                # this group's token-tile pairs, so their PSUM copies never
                # queue 8-deep in front of exp on ACT.
                if yg_prev is not None:
                    emit_proj(yg_prev, g - 1, 2 * k)
                    emit_proj(yg_prev, g - 1, 2 * k + 1)

            yg_prev = yg

        # Drain: last group's proj tiles.
        for h in range(H):
            emit_proj(yg_prev, NG - 1, h)

    if split_waits:
        _split_excess_waits(nc, mybir)
    return nc


def _get_runner(with_bias=False):
    """Build the Bass module once per bias-variant; return a cached callable
    that runs the SPMD kernel on 8 cores with device-cached weights/zeros."""
    key = ("runner", with_bias)
    if key in _STATE:
        return _STATE[key]

    import jax
    import concourse.mybir as mybir
    from concourse import bass2jax
    from jax.sharding import Mesh, NamedSharding, PartitionSpec

    try:
        from jax.experimental.shard_map import shard_map
    except ImportError:
        from jax import shard_map

    nc = _build_nc(with_bias=with_bias)
    bass2jax.install_neuronx_cc_hook()

    partition_name = (
        nc.partition_id_tensor.name if nc.partition_id_tensor else None
    )

    in_names = []
    out_names = []
    out_avals = []
    zero_shapes = []
    for alloc in nc.m.functions[0].allocations:
        if not isinstance(alloc, mybir.MemoryLocationSet):
            continue
        if not alloc.memorylocations:
            continue
        name = alloc.memorylocations[0].name
        if alloc.kind == "ExternalInput":
            if name != partition_name:
                in_names.append(name)
        elif alloc.kind == "ExternalOutput":
            out_names.append(name)
            shape = tuple(alloc.tensor_shape)
            dtype = mybir.dt.np(alloc.dtype)
            out_avals.append(jax.core.ShapedArray(shape, dtype))
            zero_shapes.append((shape, dtype))

    all_in_names = in_names + out_names
    if partition_name is not None:
        all_in_names = all_in_names + [partition_name]

    def _body(*args):
        operands = list(args)
        if partition_name is not None:
            operands.append(bass2jax.partition_id_tensor())
        outs = bass2jax._bass_exec_p.bind(
            *operands,
            out_avals=tuple(out_avals),
            in_names=tuple(all_in_names),
            out_names=tuple(out_names),
            lowering_input_output_aliases=(),
            sim_require_finite=True,
            sim_require_nnan=True,
            nc=nc,
        )
        return tuple(outs)

    devices = jax.devices()[:B]
    mesh = Mesh(np.asarray(devices), ("core",))
    n_args = len(in_names) + len(zero_shapes)
    sharded = jax.jit(
        shard_map(
            _body,
            mesh=mesh,
            in_specs=(PartitionSpec("core"),) * n_args,
            out_specs=(PartitionSpec("core"),) * len(out_names),
            check_rep=False,
        ),
        keep_unused=True,
    )
    sh = NamedSharding(mesh, PartitionSpec("core"))

    # Output scratch buffers: the kernel writes every element of "out", so
    # the initial contents are irrelevant -- stage once and reuse.
    zeros_staged = [
        jax.device_put(np.zeros((B * s[0],) + tuple(s[1:]), d), sh)
        for (s, d) in zero_shapes
    ]
    staged_cache = {}

    def _stage_cached(name, arr_concat):
        h = hashlib.blake2b(arr_concat.tobytes(), digest_size=16).digest()
        hit = staged_cache.get(name)
        if hit is not None and hit[0] == h:
            return hit[1]
        dev = jax.device_put(arr_concat, sh)
        staged_cache[name] = (h, dev)
        return dev

    def run(per_core_inputs):
        # per_core_inputs: list of B dicts name->np array (per-core shapes).
        args = []
        for nm in in_names:
            cat = np.concatenate(
                [per_core_inputs[c][nm] for c in range(B)], axis=0
            )
            if nm == "xpT":  # changes every call; skip the hash/caching
                args.append(jax.device_put(cat, sh))
            else:
                args.append(_stage_cached(nm, cat))
        out_arrs = sharded(*args, *zeros_staged)
        return [np.asarray(a) for a in out_arrs]

    parts = {
        "nc": nc,
        "body": _body,
        "mesh": mesh,
        "in_specs": (PartitionSpec("core"),) * n_args,
        "out_specs": (PartitionSpec("core"),) * len(out_names),
        "in_names": in_names,
        "out_names": out_names,
        "zero_shapes": zero_shapes,
    }
    _STATE["parts"] = parts
    _STATE[key] = run
    return run


def kernel(x, pos_32, w_qkv, b_qkv, w_proj, b_proj, resolution):
    import ml_dtypes

    bf = ml_dtypes.bfloat16

    x = np.asarray(x, dtype=np.float32)
    pos_32 = np.asarray(pos_32, dtype=np.float32)
    w_qkv = np.asarray(w_qkv, dtype=np.float32)
    b_qkv = np.asarray(b_qkv, dtype=np.float32)
    w_proj = np.asarray(w_proj, dtype=np.float32)
    b_proj = np.asarray(b_proj, dtype=np.float32)

    Bx, Nx, Cx = x.shape
    target_len = int(resolution) ** 3
    pos = _interp_linear_np(pos_32, target_len)
    xp = x + pos if pos.shape[1] == Nx else x

    # [B, C, N] bf16: transposed so lhsT tiles load contiguously.
    xpT = np.ascontiguousarray(np.transpose(xp, (0, 2, 1))).astype(bf)

    with_bias = bool(b_qkv.any() or b_proj.any())
    run = _get_runner(with_bias=with_bias)

    wq = w_qkv.astype(bf)
    bq = b_qkv.reshape(1, 3 * C).astype(bf)
    wpr = w_proj.astype(bf)
    bp = b_proj.reshape(1, C).astype(bf)
    per_core = [
        {"xpT": xpT[b], "wqkv": wq, "bqkv": bq, "wproj": wpr, "bproj": bp}
        for b in range(Bx)
    ]
    outs = run(per_core)
    return outs[0].reshape(Bx, Nx, Cx).astype(np.float32)


# revision 34
# speedup vs baseline: 1.1136x; 1.0289x over previous
"""AdaptiveAttention kernel for 8 trn2 NeuronCores — Bass/Tile implementation.

Strategy (per sharding hint): data-parallel over batch B=8, one batch element
per NeuronCore; weights replicated. Host-side preprocessing (cheap, per-token
independent): interpolate the pos table for the given resolution, add it to x,
and store x+pos transposed as bf16 so the device loads matmul lhsT tiles
directly. Device per core:

  1. qkv = (x+pos) @ w_qkv + b_qkv    -- PE matmuls (bf16)
  2. per-token 8x8 head attention     -- DVE broadcast-product + packed-2-byte
                                         tree reduction (TensorReduce has no
                                         2x mode, packed adds do), exp on ACT
  3. scrambled flatten ([N,H,D] -> transpose(1,2).reshape(N,C)) -- folded
     into a strided DMA through a DRAM bounce buffer
  4. out = y @ w_proj + b_proj        -- XBAR transpose-load of y, PE matmuls

Hardcoded problem shape: B=8, N=4096, C=512, H=8, D=64.
"""

import hashlib
import os
import sys

import numpy as np

# Persistent XLA/NEFF compilation cache: a fresh process reuses the compiled
# executable instead of re-running the Neuron compiler (~8 s).
os.environ.setdefault("JAX_COMPILATION_CACHE_DIR", "/tmp/jax_comp_cache")
os.environ.setdefault("JAX_PERSISTENT_CACHE_MIN_COMPILE_TIME_SECS", "1")

for _p in ("/opt/trn_rl_repo",):
    if _p not in sys.path:
        sys.path.append(_p)

B, N, C = 8, 4096, 512
H = 8
D = C // H  # 64
NG = 4          # token groups per core
GTOK = N // NG  # 1024 tokens per group
TT = GTOK // 128  # 8 token-tiles per group

_STATE = {}


def _interp_linear_np(pos, out_len):
    # F.interpolate(mode='linear', align_corners=False) along axis 1.
    in_len = pos.shape[1]
    if in_len == out_len:
        return pos
    scale = in_len / out_len
    coords = (np.arange(out_len, dtype=np.float64) + 0.5) * scale - 0.5
    coords = np.clip(coords, 0.0, in_len - 1)
    i0 = np.floor(coords).astype(np.int64)
    i1 = np.minimum(i0 + 1, in_len - 1)
    w = (coords - i0).astype(np.float32)[None, :, None]
    return pos[:, i0, :] * (1.0 - w) + pos[:, i1, :] * w


def _split_excess_waits(nc, mybir):
    """Walrus codegen in this container accepts at most one sync-wait per
    instruction; hoist extra waits into standalone EventSemaphore
    instructions on the same engine queue (executed in order, so semantics
    are preserved)."""
    n_split = 0
    for fn in nc.m.functions:
        for blk in fn.blocks:
            insts = blk.instructions
            out = []
            changed = False
            for inst in insts:
                si = inst.sync_info
                if si is not None and len(si.on_wait) > 1:
                    for i, w in enumerate(si.on_wait[:-1]):
                        ev = mybir.InstEventSemaphore(
                            name=f"{inst.name}-wsplit{i}"
                        )
                        ev.engine = inst.engine
                        ev.sync_info = mybir.SyncInfo(on_wait=[w], on_update=[])
                        out.append(ev)
                    inst.sync_info = mybir.SyncInfo(
                        on_wait=[si.on_wait[-1]], on_update=list(si.on_update)
                    )
                    out.append(inst)
                    changed = True
                    n_split += 1
                else:
                    out.append(inst)
            if changed:
                blk.instructions = out
    return n_split


def _build_nc(split_waits=True, with_bias=False):
    from contextlib import ExitStack

    import concourse.bass as bass
    import concourse.mybir as mybir
    import concourse.tile as tile
    from concourse.tile import add_dep_helper

    dt = mybir.dt
    bf16 = dt.bfloat16
    f16 = dt.float16
    f32 = dt.float32
    AX = mybir.AxisListType
    ALU = mybir.AluOpType
    ACTF = mybir.ActivationFunctionType

    nc = bass.Bass(num_swdge_queues=4)
    xpT = nc.declare_dram_parameter("xpT", [C, N], bf16, isOutput=False)
    wqkv = nc.declare_dram_parameter("wqkv", [C, 3 * C], bf16, isOutput=False)
    bqkv = nc.declare_dram_parameter("bqkv", [1, 3 * C], bf16, isOutput=False)
    wproj = nc.declare_dram_parameter("wproj", [C, C], bf16, isOutput=False)
    bproj = nc.declare_dram_parameter("bproj", [1, C], bf16, isOutput=False)
    out = nc.declare_dram_parameter("out", [N, C], bf16, isOutput=True)

    with tile.TileContext(nc) as tc, ExitStack() as ctx:
        const = ctx.enter_context(tc.tile_pool(name="const", bufs=1))
        xptp = ctx.enter_context(tc.tile_pool(name="xptp", bufs=2))
        qkvp = ctx.enter_context(tc.tile_pool(name="qkvp", bufs=3))
        prodp = ctx.enter_context(tc.tile_pool(name="prodp", bufs=2))
        treep = ctx.enter_context(tc.tile_pool(name="treep", bufs=2))
        smallp = ctx.enter_context(tc.tile_pool(name="smallp", bufs=3))
        outp = ctx.enter_context(tc.tile_pool(name="outp", bufs=3))
        ytp = ctx.enter_context(tc.tile_pool(name="ytp", bufs=8))
        psump = ctx.enter_context(tc.tile_pool(name="psump", bufs=2, space="PSUM"))
        dramp = ctx.enter_context(tc.tile_pool(name="dramp", bufs=2, space="DRAM"))

        # Replicated weights in SBUF. [128, cc, f] with cc = contraction chunk.
        wq3 = []
        for oc in range(3):
            wqo = const.tile([128, 4, C], bf16, tag=f"wq{oc}")
            nc.scalar.dma_start(
                out=wqo,
                in_=wqkv[:, oc * 512 : (oc + 1) * 512].rearrange(
                    "(cc p) f -> p cc f", p=128
                ),
            )
            wq3.append(wqo)
        wp = const.tile([128, 4, C], bf16)
        nc.scalar.dma_start(out=wp, in_=wproj.rearrange("(cc p) f -> p cc f", p=128))
        if with_bias:
            # Biases broadcast to all partitions (DVE cannot partition-bcast).
            bq = const.tile([128, 3 * C], bf16)
            nc.scalar.dma_start(out=bq, in_=bqkv[:].to_broadcast((128, 3 * C)))
            bp = const.tile([128, C], bf16)
            nc.scalar.dma_start(out=bp, in_=bproj[:].to_broadcast((128, C)))

        def emit_proj(yg_s, g_s, h):
            # One output row-tile: rows h*512+128*g_s .. +128.
            yt = ytp.tile([128, 4, 128], bf16, tag="yt")
            # One blocked XBAR transpose: [128, 512] -> [512, 128] delivered
            # as 4 partition blocks stacked on the free axis.
            nc.sync.dma_start_transpose(yt, yg_s[h])
            pso = psump.tile([128, C], f32, tag="proj_ps")
            for cc in range(4):
                nc.tensor.matmul(
                    pso,
                    lhsT=yt[:, cc, :],
                    rhs=wp[:, cc, :],
                    start=(cc == 0),
                    stop=(cc == 3),
                )
            osb = outp.tile([128, C], bf16, tag="osb")
            if with_bias:
                nc.vector.tensor_add(osb, pso, bp)
            else:
                osb_i = nc.scalar.activation(osb, pso, ACTF.Copy)
                if prev_exp[0] is not None:
                    add_dep_helper(
                        osb_i.ins, prev_exp[0].ins, sync=False,
                        reason="exp before proj copy on ACT",
                    )
            nc.gpsimd.dma_start(
                out=out[h * 512 + 128 * g_s : h * 512 + 128 * (g_s + 1), :],
                in_=osb,
            )

        yg_prev = None
        prev_exp = [None]
        for g in range(NG):
            # x+pos transposed slab for this group, in two half-slabs so
            # the first token-pair's matmuls don't wait for the full 2 MB.
            xpt_h = []
            for hh in range(2):
                xh = xptp.tile([128, 4, GTOK // 2], bf16, tag=f"xpt{hh}")
                nc.gpsimd.dma_start(
                    out=xh,
                    in_=xpT.rearrange("(cc p) n -> p cc n", p=128)[
                        :,
                        :,
                        g * GTOK + hh * (GTOK // 2) : g * GTOK
                        + (hh + 1) * (GTOK // 2),
                    ],
                )
                xpt_h.append(xh)
            # Scrambled attention output bounce buffer (y rows for this group).
            yg = dramp.tile([H, 128, C], bf16, tag="yg")

            for k in range(TT // 2):
                # Two 128-token subtiles share each DVE instruction (pair
                # dim on the free axis) to halve per-op fixed overheads.
                qk2 = qkvp.tile([128, 2, 2 * C], bf16, tag="qk")
                vt2 = qkvp.tile([128, 2, C], bf16, tag="vt")
                for r in range(2):
                    k2 = 2 * k + r
                    ps = psump.tile([128, 3 * C], f32, tag="qkv_ps")
                    for oc in range(3):
                        for cc in range(4):
                            nc.tensor.matmul(
                                ps[:, oc * 512 : (oc + 1) * 512],
                                lhsT=xpt_h[k2 // 4][:, cc, (k2 % 4) * 128 : (k2 % 4 + 1) * 128],
                                rhs=wq3[oc][:, cc, :],
                                start=(cc == 0),
                                stop=(cc == 3),
                            )
                    vt_jd = vt2[:, r, :].rearrange("p (d j) -> p j d", d=D)
                    vsrc = ps[:, 2 * C : 3 * C].rearrange("p (j d) -> p j d", j=H)
                    if with_bias:
                        nc.vector.tensor_add(
                            qk2[:, r, :], ps[:, 0 : 2 * C], bq[:, 0 : 2 * C]
                        )
                        nc.vector.tensor_add(
                            vt_jd,
                            vsrc,
                            bq[:, 2 * C : 3 * C].rearrange(
                                "p (j d) -> p j d", j=H
                            ),
                        )
                    else:
                        qk_i = nc.scalar.activation(
                            qk2[:, r, :], ps[:, 0 : 2 * C], ACTF.Copy
                        )
                        nc.scalar.activation(vt_jd, vsrc, ACTF.Copy)
                        if r == 0 and prev_exp[0] is not None:
                            # Keep the previous pair's exp ahead of these
                            # PE-gated copies in the in-order ACT queue.
                            add_dep_helper(
                                qk_i.ins, prev_exp[0].ins, sync=False,
                                reason="exp before next-pair ACT copies",
                            )

                # --- per-token 8x8 head attention (paired) ---
                q = qk2[:, :, 0:C].rearrange("p r (i d) -> p r i d", i=H)
                kk = qk2[:, :, C : 2 * C].rearrange("p r (j d) -> p r j d", j=H)

                p1 = prodp.tile([128, 2, H, H, D], bf16, tag="p1")
                for r in range(2):
                    nc.vector.tensor_mul(
                        p1[:, r],
                        q[:, r, :, None, :].to_broadcast((128, H, H, D)),
                        kk[:, r, None, :, :].to_broadcast((128, H, H, D)),
                    )
                p1f = p1.rearrange("p r i j d -> p r (i j) d")
                t32 = treep.tile([128, 2, H * H, 32], f16, tag="t32")
                nc.vector.tensor_add(t32, p1f[:, :, :, 0:32], p1f[:, :, :, 32:64])
                t16 = treep.tile([128, 2, H * H, 16], f16, tag="t16")
                nc.vector.tensor_add(t16, t32[:, :, :, 0:16], t32[:, :, :, 16:32])
                t8 = treep.tile([128, 2, H * H, 8], f16, tag="t8")
                nc.vector.tensor_add(t8, t16[:, :, :, 0:8], t16[:, :, :, 8:16])
                t4 = treep.tile([128, 2, H * H, 4], f16, tag="t4")
                nc.vector.tensor_add(t4, t8[:, :, :, 0:4], t8[:, :, :, 4:8])
                t2 = treep.tile([128, 2, H * H, 2], f16, tag="t2")
                nc.vector.tensor_add(t2, t4[:, :, :, 0:2], t4[:, :, :, 2:4])
                logits = smallp.tile([128, 2, H * H], f16, tag="logits")
                nc.vector.tensor_add(logits, t2[:, :, :, 0], t2[:, :, :, 1])

                # No max-subtraction needed: probs are normalized (pn <= 1)
                # before the fp16 AV tree, and the raw exponentials (logits/8
                # is O(+-30)) stay well inside bf16/fp32 range; softmax is
                # shift-invariant.
                probs = smallp.tile([128, 2, H * H], bf16, tag="probs")
                exp_i = nc.scalar.activation(
                    probs, logits, ACTF.Exp, scale=1.0 / np.sqrt(D)
                )
                prev_exp[0] = exp_i
                sums = smallp.tile([128, 2, H], f32, tag="sums")
                nc.vector.tensor_reduce(
                    sums,
                    probs.rearrange("p r (i j) -> p r i j", i=H),
                    axis=AX.X,
                    op=ALU.add,
                )
                recip = smallp.tile([128, 2, H], f32, tag="recip")
                nc.vector.reciprocal(recip, sums)
                pn = smallp.tile([128, 2, H * H], bf16, tag="pn")
                nc.vector.tensor_mul(
                    pn.rearrange("p r (i j) -> p r i j", i=H),
                    probs.rearrange("p r (i j) -> p r i j", i=H),
                    recip[:, :, :, None].to_broadcast((128, 2, H, H)),
                )

                p2 = prodp.tile([128, 2, H, D, H], bf16, tag="p2")
                for r in range(2):
                    nc.vector.tensor_mul(
                        p2[:, r],
                        pn.rearrange("p r (i j) -> p r i j", i=H)[
                            :, r, :, None, :
                        ].to_broadcast((128, H, D, H)),
                        vt2[:, r, :].rearrange("p (d j) -> p d j", d=D)[
                            :, None, :, :
                        ].to_broadcast((128, H, D, H)),
                    )
                p2f = p2.rearrange("p r i d j -> p r (i d) j")
                a4 = treep.tile([128, 2, H * D, 4], f16, tag="a4")
                nc.vector.tensor_add(a4, p2f[:, :, :, 0:4], p2f[:, :, :, 4:8])
                a2 = treep.tile([128, 2, H * D, 2], f16, tag="a2")
                nc.vector.tensor_add(a2, a4[:, :, :, 0:2], a4[:, :, :, 2:4])
                outsb = outp.tile([128, 2, C], bf16, tag="outsb")
                nc.vector.tensor_add(outsb, a2[:, :, :, 0], a2[:, :, :, 1])

                # --- scrambled flatten to DRAM bounce buffer ---
                # y[h*512 + 16k + s, t*64 + d] = out[8s + t, h*64 + d]
                for r in range(2):
                    k2 = 2 * k + r
                    dst = yg[:, 16 * k2 : 16 * (k2 + 1), :].rearrange(
                        "h s (t d) -> s t h d", t=8
                    )
                    nc.gpsimd.dma_start(out=dst, in_=outsb[:, r, :])

                # Software pipeline: previous group's proj tiles between
                # this group's token-tile pairs, so their PSUM copies never
                # queue 8-deep in front of exp on ACT.
                if yg_prev is not None:
                    emit_proj(yg_prev, g - 1, 2 * k)
                    emit_proj(yg_prev, g - 1, 2 * k + 1)

            yg_prev = yg

        # Drain: last group's proj tiles.
        for h in range(H):
            emit_proj(yg_prev, NG - 1, h)

    if split_waits:
        _split_excess_waits(nc, mybir)
    return nc


def _get_runner(with_bias=False):
    """Build the Bass module once per bias-variant; return a cached callable
    that runs the SPMD kernel on 8 cores with device-cached weights/zeros."""
    key = ("runner", with_bias)
    if key in _STATE:
        return _STATE[key]

    import jax
    import concourse.mybir as mybir
    from concourse import bass2jax
    from jax.sharding import Mesh, NamedSharding, PartitionSpec

    try:
        from jax.experimental.shard_map import shard_map
    except ImportError:
        from jax import shard_map

    nc = _build_nc(with_bias=with_bias)
    bass2jax.install_neuronx_cc_hook()

    partition_name = (
        nc.partition_id_tensor.name if nc.partition_id_tensor else None
    )

    in_names = []
    out_names = []
    out_avals = []
    zero_shapes = []
    for alloc in nc.m.functions[0].allocations:
        if not isinstance(alloc, mybir.MemoryLocationSet):
            continue
        if not alloc.memorylocations:
            continue
        name = alloc.memorylocations[0].name
        if alloc.kind == "ExternalInput":
            if name != partition_name:
                in_names.append(name)
        elif alloc.kind == "ExternalOutput":
            out_names.append(name)
            shape = tuple(alloc.tensor_shape)
            dtype = mybir.dt.np(alloc.dtype)
            out_avals.append(jax.core.ShapedArray(shape, dtype))
            zero_shapes.append((shape, dtype))

    all_in_names = in_names + out_names
    if partition_name is not None:
        all_in_names = all_in_names + [partition_name]

    def _body(*args):
        operands = list(args)
        if partition_name is not None:
            operands.append(bass2jax.partition_id_tensor())
        outs = bass2jax._bass_exec_p.bind(
            *operands,
            out_avals=tuple(out_avals),
            in_names=tuple(all_in_names),
            out_names=tuple(out_names),
            lowering_input_output_aliases=(),
            sim_require_finite=True,
            sim_require_nnan=True,
            nc=nc,
        )
        return tuple(outs)

    devices = jax.devices()[:B]
    mesh = Mesh(np.asarray(devices), ("core",))
    n_args = len(in_names) + len(zero_shapes)
    sharded = jax.jit(
        shard_map(
            _body,
            mesh=mesh,
            in_specs=(PartitionSpec("core"),) * n_args,
            out_specs=(PartitionSpec("core"),) * len(out_names),
            check_rep=False,
        ),
        keep_unused=True,
    )
    sh = NamedSharding(mesh, PartitionSpec("core"))

    # Output scratch buffers: the kernel writes every element of "out", so
    # the initial contents are irrelevant -- stage once and reuse.
    zeros_staged = [
        jax.device_put(np.zeros((B * s[0],) + tuple(s[1:]), d), sh)
        for (s, d) in zero_shapes
    ]
    staged_cache = {}

    def _stage_cached(name, arr_concat):
        h = hashlib.blake2b(arr_concat.tobytes(), digest_size=16).digest()
        hit = staged_cache.get(name)
        if hit is not None and hit[0] == h:
            return hit[1]
        dev = jax.device_put(arr_concat, sh)
        staged_cache[name] = (h, dev)
        return dev

    def run(per_core_inputs):
        # per_core_inputs: list of B dicts name->np array (per-core shapes).
        args = []
        for nm in in_names:
            cat = np.concatenate(
                [per_core_inputs[c][nm] for c in range(B)], axis=0
            )
            if nm == "xpT":  # changes every call; skip the hash/caching
                args.append(jax.device_put(cat, sh))
            else:
                args.append(_stage_cached(nm, cat))
        out_arrs = sharded(*args, *zeros_staged)
        return [np.asarray(a) for a in out_arrs]

    parts = {
        "nc": nc,
        "body": _body,
        "mesh": mesh,
        "in_specs": (PartitionSpec("core"),) * n_args,
        "out_specs": (PartitionSpec("core"),) * len(out_names),
        "in_names": in_names,
        "out_names": out_names,
        "zero_shapes": zero_shapes,
    }
    _STATE["parts"] = parts
    _STATE[key] = run
    return run


def kernel(x, pos_32, w_qkv, b_qkv, w_proj, b_proj, resolution):
    import ml_dtypes

    bf = ml_dtypes.bfloat16

    x = np.asarray(x, dtype=np.float32)
    pos_32 = np.asarray(pos_32, dtype=np.float32)
    w_qkv = np.asarray(w_qkv, dtype=np.float32)
    b_qkv = np.asarray(b_qkv, dtype=np.float32)
    w_proj = np.asarray(w_proj, dtype=np.float32)
    b_proj = np.asarray(b_proj, dtype=np.float32)

    Bx, Nx, Cx = x.shape
    target_len = int(resolution) ** 3
    pos = _interp_linear_np(pos_32, target_len)
    xp = x + pos if pos.shape[1] == Nx else x

    # [B, C, N] bf16: transposed so lhsT tiles load contiguously.
    xpT = np.ascontiguousarray(np.transpose(xp, (0, 2, 1))).astype(bf)

    with_bias = bool(b_qkv.any() or b_proj.any())
    run = _get_runner(with_bias=with_bias)

    wq = w_qkv.astype(bf)
    bq = b_qkv.reshape(1, 3 * C).astype(bf)
    wpr = w_proj.astype(bf)
    bp = b_proj.reshape(1, C).astype(bf)
    per_core = [
        {"xpT": xpT[b], "wqkv": wq, "bqkv": bq, "wproj": wpr, "bproj": bp}
        for b in range(Bx)
    ]
    outs = run(per_core)
    return outs[0].reshape(Bx, Nx, Cx).astype(np.float32)


# revision 35
# speedup vs baseline: 1.1344x; 1.0187x over previous
"""AdaptiveAttention kernel for 8 trn2 NeuronCores — Bass/Tile implementation.

Strategy (per sharding hint): data-parallel over batch B=8, one batch element
per NeuronCore; weights replicated. Host-side preprocessing (cheap, per-token
independent): interpolate the pos table for the given resolution, add it to x,
and store x+pos transposed as bf16 so the device loads matmul lhsT tiles
directly. Device per core:

  1. qkv = (x+pos) @ w_qkv + b_qkv    -- PE matmuls (bf16)
  2. per-token 8x8 head attention     -- DVE broadcast-product + packed-2-byte
                                         tree reduction (TensorReduce has no
                                         2x mode, packed adds do), exp on ACT
  3. scrambled flatten ([N,H,D] -> transpose(1,2).reshape(N,C)) -- folded
     into a strided DMA through a DRAM bounce buffer
  4. out = y @ w_proj + b_proj        -- XBAR transpose-load of y, PE matmuls

Hardcoded problem shape: B=8, N=4096, C=512, H=8, D=64.
"""

import hashlib
import os
import sys

import numpy as np

# Persistent XLA/NEFF compilation cache: a fresh process reuses the compiled
# executable instead of re-running the Neuron compiler (~8 s).
os.environ.setdefault("JAX_COMPILATION_CACHE_DIR", "/tmp/jax_comp_cache")
os.environ.setdefault("JAX_PERSISTENT_CACHE_MIN_COMPILE_TIME_SECS", "1")

for _p in ("/opt/trn_rl_repo",):
    if _p not in sys.path:
        sys.path.append(_p)

B, N, C = 8, 4096, 512
H = 8
D = C // H  # 64
NG = 4          # token groups per core
GTOK = N // NG  # 1024 tokens per group
TT = GTOK // 128  # 8 token-tiles per group

_STATE = {}


def _interp_linear_np(pos, out_len):
    # F.interpolate(mode='linear', align_corners=False) along axis 1.
    in_len = pos.shape[1]
    if in_len == out_len:
        return pos
    scale = in_len / out_len
    coords = (np.arange(out_len, dtype=np.float64) + 0.5) * scale - 0.5
    coords = np.clip(coords, 0.0, in_len - 1)
    i0 = np.floor(coords).astype(np.int64)
    i1 = np.minimum(i0 + 1, in_len - 1)
    w = (coords - i0).astype(np.float32)[None, :, None]
    return pos[:, i0, :] * (1.0 - w) + pos[:, i1, :] * w


def _split_excess_waits(nc, mybir):
    """Walrus codegen in this container accepts at most one sync-wait per
    instruction; hoist extra waits into standalone EventSemaphore
    instructions on the same engine queue (executed in order, so semantics
    are preserved)."""
    n_split = 0
    for fn in nc.m.functions:
        for blk in fn.blocks:
            insts = blk.instructions
            out = []
            changed = False
            for inst in insts:
                si = inst.sync_info
                if si is not None and len(si.on_wait) > 1:
                    for i, w in enumerate(si.on_wait[:-1]):
                        ev = mybir.InstEventSemaphore(
                            name=f"{inst.name}-wsplit{i}"
                        )
                        ev.engine = inst.engine
                        ev.sync_info = mybir.SyncInfo(on_wait=[w], on_update=[])
                        out.append(ev)
                    inst.sync_info = mybir.SyncInfo(
                        on_wait=[si.on_wait[-1]], on_update=list(si.on_update)
                    )
                    out.append(inst)
                    changed = True
                    n_split += 1
                else:
                    out.append(inst)
            if changed:
                blk.instructions = out
    return n_split


def _build_nc(split_waits=True, with_bias=False):
    from contextlib import ExitStack

    import concourse.bass as bass
    import concourse.mybir as mybir
    import concourse.tile as tile
    from concourse.tile import add_dep_helper

    dt = mybir.dt
    bf16 = dt.bfloat16
    f16 = dt.float16
    f32 = dt.float32
    AX = mybir.AxisListType
    ALU = mybir.AluOpType
    ACTF = mybir.ActivationFunctionType

    nc = bass.Bass(num_swdge_queues=4)
    xpT = nc.declare_dram_parameter("xpT", [C, N], bf16, isOutput=False)
    wqkv = nc.declare_dram_parameter("wqkv", [C, 3 * C], bf16, isOutput=False)
    bqkv = nc.declare_dram_parameter("bqkv", [1, 3 * C], bf16, isOutput=False)
    wproj = nc.declare_dram_parameter("wproj", [C, C], bf16, isOutput=False)
    bproj = nc.declare_dram_parameter("bproj", [1, C], bf16, isOutput=False)
    out = nc.declare_dram_parameter("out", [N, C], bf16, isOutput=True)

    with tile.TileContext(nc) as tc, ExitStack() as ctx:
        const = ctx.enter_context(tc.tile_pool(name="const", bufs=1))
        xptp = ctx.enter_context(tc.tile_pool(name="xptp", bufs=2))
        qkvp = ctx.enter_context(tc.tile_pool(name="qkvp", bufs=3))
        prodp = ctx.enter_context(tc.tile_pool(name="prodp", bufs=2))
        treep = ctx.enter_context(tc.tile_pool(name="treep", bufs=2))
        smallp = ctx.enter_context(tc.tile_pool(name="smallp", bufs=3))
        outp = ctx.enter_context(tc.tile_pool(name="outp", bufs=3))
        ytp = ctx.enter_context(tc.tile_pool(name="ytp", bufs=8))
        psump = ctx.enter_context(tc.tile_pool(name="psump", bufs=2, space="PSUM"))
        dramp = ctx.enter_context(tc.tile_pool(name="dramp", bufs=2, space="DRAM"))

        # Replicated weights in SBUF. [128, cc, f] with cc = contraction chunk.
        wq3 = []
        for oc in range(3):
            wqo = const.tile([128, 4, C], bf16, tag=f"wq{oc}")
            nc.scalar.dma_start(
                out=wqo,
                in_=wqkv[:, oc * 512 : (oc + 1) * 512].rearrange(
                    "(cc p) f -> p cc f", p=128
                ),
            )
            wq3.append(wqo)
        wp = const.tile([128, 4, C], bf16)
        nc.scalar.dma_start(out=wp, in_=wproj.rearrange("(cc p) f -> p cc f", p=128))
        if with_bias:
            # Biases broadcast to all partitions (DVE cannot partition-bcast).
            bq = const.tile([128, 3 * C], bf16)
            nc.scalar.dma_start(out=bq, in_=bqkv[:].to_broadcast((128, 3 * C)))
            bp = const.tile([128, C], bf16)
            nc.scalar.dma_start(out=bp, in_=bproj[:].to_broadcast((128, C)))

        def emit_proj(yg_s, g_s, h):
            # One output row-tile: rows h*512+128*g_s .. +128.
            yt = ytp.tile([128, 4, 128], bf16, tag="yt")
            # One blocked XBAR transpose: [128, 512] -> [512, 128] delivered
            # as 4 partition blocks stacked on the free axis. Mid-kernel they
            # all ride the SP queue (scalar-queue transposes would block ACT
            # compute); the LAST group's burst has no ACT compute behind it,
            # so split it across both HWDGE queues to halve the tail.
            eng = nc.scalar if (g_s == NG - 1 and h % 2 == 1) else nc.sync
            eng.dma_start_transpose(yt, yg_s[h])
            pso = psump.tile([128, C], f32, tag="proj_ps")
            for cc in range(4):
                nc.tensor.matmul(
                    pso,
                    lhsT=yt[:, cc, :],
                    rhs=wp[:, cc, :],
                    start=(cc == 0),
                    stop=(cc == 3),
                )
            osb = outp.tile([128, C], bf16, tag="osb")
            if with_bias:
                nc.vector.tensor_add(osb, pso, bp)
            else:
                osb_i = nc.scalar.activation(osb, pso, ACTF.Copy)
                if prev_exp[0] is not None:
                    add_dep_helper(
                        osb_i.ins, prev_exp[0].ins, sync=False,
                        reason="exp before proj copy on ACT",
                    )
            nc.gpsimd.dma_start(
                out=out[h * 512 + 128 * g_s : h * 512 + 128 * (g_s + 1), :],
                in_=osb,
            )

        yg_prev = None
        prev_exp = [None]
        for g in range(NG):
            # x+pos transposed slab for this group, in two half-slabs so
            # the first token-pair's matmuls don't wait for the full 2 MB.
            xpt_h = []
            for hh in range(2):
                xh = xptp.tile([128, 4, GTOK // 2], bf16, tag=f"xpt{hh}")
                nc.gpsimd.dma_start(
                    out=xh,
                    in_=xpT.rearrange("(cc p) n -> p cc n", p=128)[
                        :,
                        :,
                        g * GTOK + hh * (GTOK // 2) : g * GTOK
                        + (hh + 1) * (GTOK // 2),
                    ],
                )
                xpt_h.append(xh)
            # Scrambled attention output bounce buffer (y rows for this group).
            yg = dramp.tile([H, 128, C], bf16, tag="yg")

            for k in range(TT // 2):
                # Two 128-token subtiles share each DVE instruction (pair
                # dim on the free axis) to halve per-op fixed overheads.
                qk2 = qkvp.tile([128, 2, 2 * C], bf16, tag="qk")
                vt2 = qkvp.tile([128, 2, C], bf16, tag="vt")
                for r in range(2):
                    k2 = 2 * k + r
                    ps = psump.tile([128, 3 * C], f32, tag="qkv_ps")
                    for oc in range(3):
                        for cc in range(4):
                            nc.tensor.matmul(
                                ps[:, oc * 512 : (oc + 1) * 512],
                                lhsT=xpt_h[k2 // 4][:, cc, (k2 % 4) * 128 : (k2 % 4 + 1) * 128],
                                rhs=wq3[oc][:, cc, :],
                                start=(cc == 0),
                                stop=(cc == 3),
                            )
                    vt_jd = vt2[:, r, :].rearrange("p (d j) -> p j d", d=D)
                    vsrc = ps[:, 2 * C : 3 * C].rearrange("p (j d) -> p j d", j=H)
                    if with_bias:
                        nc.vector.tensor_add(
                            qk2[:, r, :], ps[:, 0 : 2 * C], bq[:, 0 : 2 * C]
                        )
                        nc.vector.tensor_add(
                            vt_jd,
                            vsrc,
                            bq[:, 2 * C : 3 * C].rearrange(
                                "p (j d) -> p j d", j=H
                            ),
                        )
                    else:
                        qk_i = nc.scalar.activation(
                            qk2[:, r, :], ps[:, 0 : 2 * C], ACTF.Copy
                        )
                        nc.scalar.activation(vt_jd, vsrc, ACTF.Copy)
                        if r == 0 and prev_exp[0] is not None:
                            # Keep the previous pair's exp ahead of these
                            # PE-gated copies in the in-order ACT queue.
                            add_dep_helper(
                                qk_i.ins, prev_exp[0].ins, sync=False,
                                reason="exp before next-pair ACT copies",
                            )

                # --- per-token 8x8 head attention (paired) ---
                q = qk2[:, :, 0:C].rearrange("p r (i d) -> p r i d", i=H)
                kk = qk2[:, :, C : 2 * C].rearrange("p r (j d) -> p r j d", j=H)

                p1 = prodp.tile([128, 2, H, H, D], bf16, tag="p1")
                for r in range(2):
                    nc.vector.tensor_mul(
                        p1[:, r],
                        q[:, r, :, None, :].to_broadcast((128, H, H, D)),
                        kk[:, r, None, :, :].to_broadcast((128, H, H, D)),
                    )
                p1f = p1.rearrange("p r i j d -> p r (i j) d")
                t32 = treep.tile([128, 2, H * H, 32], f16, tag="t32")
                nc.vector.tensor_add(t32, p1f[:, :, :, 0:32], p1f[:, :, :, 32:64])
                t16 = treep.tile([128, 2, H * H, 16], f16, tag="t16")
                nc.vector.tensor_add(t16, t32[:, :, :, 0:16], t32[:, :, :, 16:32])
                t8 = treep.tile([128, 2, H * H, 8], f16, tag="t8")
                nc.vector.tensor_add(t8, t16[:, :, :, 0:8], t16[:, :, :, 8:16])
                t4 = treep.tile([128, 2, H * H, 4], f16, tag="t4")
                nc.vector.tensor_add(t4, t8[:, :, :, 0:4], t8[:, :, :, 4:8])
                t2 = treep.tile([128, 2, H * H, 2], f16, tag="t2")
                nc.vector.tensor_add(t2, t4[:, :, :, 0:2], t4[:, :, :, 2:4])
                logits = smallp.tile([128, 2, H * H], f16, tag="logits")
                nc.vector.tensor_add(logits, t2[:, :, :, 0], t2[:, :, :, 1])

                # No max-subtraction needed: probs are normalized (pn <= 1)
                # before the fp16 AV tree, and the raw exponentials (logits/8
                # is O(+-30)) stay well inside bf16/fp32 range; softmax is
                # shift-invariant.
                probs = smallp.tile([128, 2, H * H], bf16, tag="probs")
                exp_i = nc.scalar.activation(
                    probs, logits, ACTF.Exp, scale=1.0 / np.sqrt(D)
                )
                prev_exp[0] = exp_i
                sums = smallp.tile([128, 2, H], f32, tag="sums")
                nc.vector.tensor_reduce(
                    sums,
                    probs.rearrange("p r (i j) -> p r i j", i=H),
                    axis=AX.X,
                    op=ALU.add,
                )
                recip = smallp.tile([128, 2, H], f32, tag="recip")
                nc.vector.reciprocal(recip, sums)
                pn = smallp.tile([128, 2, H * H], bf16, tag="pn")
                nc.vector.tensor_mul(
                    pn.rearrange("p r (i j) -> p r i j", i=H),
                    probs.rearrange("p r (i j) -> p r i j", i=H),
                    recip[:, :, :, None].to_broadcast((128, 2, H, H)),
                )

                p2 = prodp.tile([128, 2, H, D, H], bf16, tag="p2")
                for r in range(2):
                    nc.vector.tensor_mul(
                        p2[:, r],
                        pn.rearrange("p r (i j) -> p r i j", i=H)[
                            :, r, :, None, :
                        ].to_broadcast((128, H, D, H)),
                        vt2[:, r, :].rearrange("p (d j) -> p d j", d=D)[
                            :, None, :, :
                        ].to_broadcast((128, H, D, H)),
                    )
                p2f = p2.rearrange("p r i d j -> p r (i d) j")
                a4 = treep.tile([128, 2, H * D, 4], f16, tag="a4")
                nc.vector.tensor_add(a4, p2f[:, :, :, 0:4], p2f[:, :, :, 4:8])
                a2 = treep.tile([128, 2, H * D, 2], f16, tag="a2")
                nc.vector.tensor_add(a2, a4[:, :, :, 0:2], a4[:, :, :, 2:4])
                outsb = outp.tile([128, 2, C], bf16, tag="outsb")
                nc.vector.tensor_add(outsb, a2[:, :, :, 0], a2[:, :, :, 1])

                # --- scrambled flatten to DRAM bounce buffer ---
                # y[h*512 + 16k + s, t*64 + d] = out[8s + t, h*64 + d]
                for r in range(2):
                    k2 = 2 * k + r
                    dst = yg[:, 16 * k2 : 16 * (k2 + 1), :].rearrange(
                        "h s (t d) -> s t h d", t=8
                    )
                    nc.gpsimd.dma_start(out=dst, in_=outsb[:, r, :])

                # Software pipeline: previous group's proj tiles between
                # this group's token-tile pairs, so their PSUM copies never
                # queue 8-deep in front of exp on ACT.
                if yg_prev is not None:
                    emit_proj(yg_prev, g - 1, 2 * k)
                    emit_proj(yg_prev, g - 1, 2 * k + 1)

            yg_prev = yg

        # Drain: last group's proj tiles.
        for h in range(H):
            emit_proj(yg_prev, NG - 1, h)

    if split_waits:
        _split_excess_waits(nc, mybir)
    return nc


def _get_runner(with_bias=False):
    """Build the Bass module once per bias-variant; return a cached callable
    that runs the SPMD kernel on 8 cores with device-cached weights/zeros."""
    key = ("runner", with_bias)
    if key in _STATE:
        return _STATE[key]

    import jax
    import concourse.mybir as mybir
    from concourse import bass2jax
    from jax.sharding import Mesh, NamedSharding, PartitionSpec

    try:
        from jax.experimental.shard_map import shard_map
    except ImportError:
        from jax import shard_map

    nc = _build_nc(with_bias=with_bias)
    bass2jax.install_neuronx_cc_hook()

    partition_name = (
        nc.partition_id_tensor.name if nc.partition_id_tensor else None
    )

    in_names = []
    out_names = []
    out_avals = []
    zero_shapes = []
    for alloc in nc.m.functions[0].allocations:
        if not isinstance(alloc, mybir.MemoryLocationSet):
            continue
        if not alloc.memorylocations:
            continue
        name = alloc.memorylocations[0].name
        if alloc.kind == "ExternalInput":
            if name != partition_name:
                in_names.append(name)
        elif alloc.kind == "ExternalOutput":
            out_names.append(name)
            shape = tuple(alloc.tensor_shape)
            dtype = mybir.dt.np(alloc.dtype)
            out_avals.append(jax.core.ShapedArray(shape, dtype))
            zero_shapes.append((shape, dtype))

    all_in_names = in_names + out_names
    if partition_name is not None:
        all_in_names = all_in_names + [partition_name]

    def _body(*args):
        operands = list(args)
        if partition_name is not None:
            operands.append(bass2jax.partition_id_tensor())
        outs = bass2jax._bass_exec_p.bind(
            *operands,
            out_avals=tuple(out_avals),
            in_names=tuple(all_in_names),
            out_names=tuple(out_names),
            lowering_input_output_aliases=(),
            sim_require_finite=True,
            sim_require_nnan=True,
            nc=nc,
        )
        return tuple(outs)

    devices = jax.devices()[:B]
    mesh = Mesh(np.asarray(devices), ("core",))
    n_args = len(in_names) + len(zero_shapes)
    sharded = jax.jit(
        shard_map(
            _body,
            mesh=mesh,
            in_specs=(PartitionSpec("core"),) * n_args,
            out_specs=(PartitionSpec("core"),) * len(out_names),
            check_rep=False,
        ),
        keep_unused=True,
    )
    sh = NamedSharding(mesh, PartitionSpec("core"))

    # Output scratch buffers: the kernel writes every element of "out", so
    # the initial contents are irrelevant -- stage once and reuse.
    zeros_staged = [
        jax.device_put(np.zeros((B * s[0],) + tuple(s[1:]), d), sh)
        for (s, d) in zero_shapes
    ]
    staged_cache = {}

    def _stage_cached(name, arr_concat):
        h = hashlib.blake2b(arr_concat.tobytes(), digest_size=16).digest()
        hit = staged_cache.get(name)
        if hit is not None and hit[0] == h:
            return hit[1]
        dev = jax.device_put(arr_concat, sh)
        staged_cache[name] = (h, dev)
        return dev

    def run(per_core_inputs):
        # per_core_inputs: list of B dicts name->np array (per-core shapes).
        args = []
        for nm in in_names:
            cat = np.concatenate(
                [per_core_inputs[c][nm] for c in range(B)], axis=0
            )
            if nm == "xpT":  # changes every call; skip the hash/caching
                args.append(jax.device_put(cat, sh))
            else:
                args.append(_stage_cached(nm, cat))
        out_arrs = sharded(*args, *zeros_staged)
        return [np.asarray(a) for a in out_arrs]

    parts = {
        "nc": nc,
        "body": _body,
        "mesh": mesh,
        "in_specs": (PartitionSpec("core"),) * n_args,
        "out_specs": (PartitionSpec("core"),) * len(out_names),
        "in_names": in_names,
        "out_names": out_names,
        "zero_shapes": zero_shapes,
    }
    _STATE["parts"] = parts
    _STATE[key] = run
    return run


def kernel(x, pos_32, w_qkv, b_qkv, w_proj, b_proj, resolution):
    import ml_dtypes

    bf = ml_dtypes.bfloat16

    x = np.asarray(x, dtype=np.float32)
    pos_32 = np.asarray(pos_32, dtype=np.float32)
    w_qkv = np.asarray(w_qkv, dtype=np.float32)
    b_qkv = np.asarray(b_qkv, dtype=np.float32)
    w_proj = np.asarray(w_proj, dtype=np.float32)
    b_proj = np.asarray(b_proj, dtype=np.float32)

    Bx, Nx, Cx = x.shape
    target_len = int(resolution) ** 3
    pos = _interp_linear_np(pos_32, target_len)
    xp = x + pos if pos.shape[1] == Nx else x

    # [B, C, N] bf16: transposed so lhsT tiles load contiguously.
    xpT = np.ascontiguousarray(np.transpose(xp, (0, 2, 1))).astype(bf)

    with_bias = bool(b_qkv.any() or b_proj.any())
    run = _get_runner(with_bias=with_bias)

    wq = w_qkv.astype(bf)
    bq = b_qkv.reshape(1, 3 * C).astype(bf)
    wpr = w_proj.astype(bf)
    bp = b_proj.reshape(1, C).astype(bf)
    per_core = [
        {"xpT": xpT[b], "wqkv": wq, "bqkv": bq, "wproj": wpr, "bproj": bp}
        for b in range(Bx)
    ]
    outs = run(per_core)
    return outs[0].reshape(Bx, Nx, Cx).astype(np.float32)
